# revision 6
# baseline (speedup 1.0000x reference)
"""Trainium2 Bass kernel for nn_Network_10256381903586.

Population-density LIF network RHS:
  y = [ro (N), V (N)] -> dy/dt, N = 8,000,000.

Decomposition across 8 NeuronCores (data-parallel, no collectives):
  - Each core owns a contiguous chunk of S_OWN = 2^20 grid points of both
    ro and V (total 8*2^20 >= N; tail is zero-padded).
  - Per-core inputs carry a 2-left/1-right element halo so the 4-point TVD
    stencil is uniform everywhere; global edge cells (4 elements) and the
    firing-rate feedback (a single scalar = sum(ro*H), which only affects
    output element 0) are patched on the host from per-core partial sums.
  - Layout on core: chunk viewed as [128 partitions x LW=8192] row-major
    (partition p = contiguous segment), so the stencil is a free-axis
    shift. Tiles of width W columns, each loaded with a 3-column halo.

Math notes (exact rewrites of the reference):
  - limiter(a,b) = min(0.5|a+b|, 2min(|a|,|b|))  (the reference's masked
    sequence reduces to this because its two index sets are disjoint).
  - The quartic exp argument is factored into two quadratics so the ACT
    engine's Square(scale*x+bias) evaluates most of it.
  - exp(-T^2)/(1.00000001+erf(T)) = exp(-(T^2 + ln(1.00000001+erf(T)))).
"""
import math

import numpy as np

# ---------------- problem constants ----------------
N = 8_000_000
GL = 0.1
EL = -5.0
Cm = 0.3
IEXT = 0.4
DTS = 0.5
DT = 0.1
SQ2 = math.sqrt(2.0)
SQ2PI = 0.7978845608028654
SIGMA = 0.3 / GL * math.sqrt(0.5 * GL / Cm)
COEF = 0.5 * (1.0 - DT / DTS)            # 0.4
K = 1.0 / (SIGMA * SQ2)                  # T = K * delta_V  (= 1/sqrt(3))
CC = SQ2 * K * SQ2PI                     # g = relu(CC * dVdt)
A_CONST = -GL / Cm

# quartic p(T) = C4*T^4 + ... + C0 factored: C4*(T^2+al*T+be)(T^2+ga*T+de)
C0, C1, C2, C3, C4 = 0.0061, -1.12, -0.257, -0.072, -0.0117


def _quartic_factors():
    r = np.roots([C4, C3, C2, C1, C0])
    used = [False] * 4
    quads = []
    for i in range(4):
        if used[i]:
            continue
        ri = r[i]
        if abs(ri.imag) > 1e-12:
            for j in range(i + 1, 4):
                if not used[j] and abs(r[j] - np.conj(ri)) < 1e-8:
                    used[i] = used[j] = True
                    quads.append((-(2 * ri.real), (ri * np.conj(ri)).real))
                    break
        else:
            for j in range(i + 1, 4):
                if not used[j] and abs(r[j].imag) < 1e-12:
                    used[i] = used[j] = True
                    quads.append((-(ri + r[j]).real, (ri * r[j]).real))
                    break
    (al, be), (ga, de) = quads
    return al, be, ga, de


_AL, _BE, _GA, _DE = _quartic_factors()
AL2 = _AL / 2.0
GA2 = _GA / 2.0
E1 = _BE - _AL * _AL / 4.0
E2 = _DE - _GA * _GA / 4.0

NSCAL = 5
NCORES = 8
LW = 8192                 # row length per partition
S_OWN = 128 * LW          # 2^20 owned elements per core
TOT = NCORES * S_OWN
W = 512                   # tile width (columns)


# ---------------- Bass program ----------------
def build_program(lw=LW, w=W):
    import concourse.bacc as bacc
    import concourse.mybir as mybir
    import concourse.tile as tile

    AF = mybir.ActivationFunctionType
    OP = mybir.AluOpType
    F32 = mybir.dt.float32
    nt = lw // w
    assert lw % w == 0

    nc = bacc.Bacc("TRN2", target_bir_lowering=False, debug=False)
    zin = nc.dram_tensor("zin", [2, 128, lw + 3], F32, kind="ExternalInput")
    scal = nc.dram_tensor("scal", [128, NSCAL], F32, kind="ExternalInput")
    dout = nc.dram_tensor("dout", [2, 128, lw], F32, kind="ExternalOutput")
    accout = nc.dram_tensor("accout", [128, 1], F32, kind="ExternalOutput")
    zin_ap, scal_ap = zin.ap(), scal.ap()
    zin_r = zin_ap.rearrange("q p c -> p q c")
    dout_ap, accout_ap = dout.ap(), accout.ap()

    with tile.TileContext(nc) as tc:
        with tc.tile_pool(name="main", bufs=2) as pool, \
             tc.tile_pool(name="persist", bufs=1) as pp:
            scal_sb = pp.tile([128, NSCAL], F32)
            nc.sync.dma_start(out=scal_sb[:, :], in_=scal_ap)
            b_ap = scal_sb[:, 0:1]
            invtau_ap = scal_sb[:, 1:2]
            al2_ap = scal_sb[:, 2:3]
            ga2_ap = scal_sb[:, 3:4]
            one_ap = scal_sb[:, 4:5]
            acc = pp.tile([128, nt], F32)

            for t in range(nt):
                c0 = t * w
                z2 = pool.tile([128, 2, w + 3], F32)
                nc.sync.dma_start(out=z2[:, :, :], in_=zin_r[:, :, c0:c0 + w + 3])
                Vo = z2[:, 1, 2:w + 2]
                ro_o = z2[:, 0, 2:w + 2]

                # ---- H chain (V only, width w) ----
                dVdt = pool.tile([128, w], F32)
                nc.scalar.activation(dVdt[:, :], Vo, AF.Identity,
                                     bias=b_ap, scale=A_CONST)
                delta_V = pool.tile([128, w], F32)
                nc.vector.tensor_scalar(delta_V[:, :], Vo, -1.0, -1.0,
                                        OP.mult, OP.max)
                T2 = pool.tile([128, w], F32)
                nc.scalar.activation(T2[:, :], delta_V[:, :], AF.Square, scale=K)
                Q1 = pool.tile([128, w], F32)
                nc.scalar.activation(Q1[:, :], delta_V[:, :], AF.Square,
                                     bias=al2_ap, scale=K)
                Q2 = pool.tile([128, w], F32)
                nc.scalar.activation(Q2[:, :], delta_V[:, :], AF.Square,
                                     bias=ga2_ap, scale=K)
                U2 = pool.tile([128, w], F32)
                nc.vector.tensor_scalar(U2[:, :], Q2[:, :], E2, None, OP.add)
                PT = pool.tile([128, w], F32)
                nc.vector.scalar_tensor_tensor(PT[:, :], Q1[:, :], E1, U2[:, :],
                                               OP.add, OP.mult)
                Aex = pool.tile([128, w], F32)
                nc.scalar.activation(Aex[:, :], PT[:, :], AF.Exp, scale=C4)
                erf = pool.tile([128, w], F32)
                nc.scalar.activation(erf[:, :], delta_V[:, :], AF.Erf, scale=K)
                lnden = pool.tile([128, w], F32)
                nc.scalar.activation(lnden[:, :], erf[:, :], AF.Ln,
                                     bias=one_ap)
                r1 = pool.tile([128, w], F32)
                nc.vector.tensor_add(r1[:, :], T2[:, :], lnden[:, :])
                Fden = pool.tile([128, w], F32)
                nc.scalar.activation(Fden[:, :], r1[:, :], AF.Exp, scale=-1.0)
                g = pool.tile([128, w], F32)
                nc.scalar.activation(g[:, :], dVdt[:, :], AF.Relu, scale=CC)
                m1 = pool.tile([128, w], F32)
                nc.vector.tensor_mul(m1[:, :], g[:, :], Fden[:, :])
                Hv = pool.tile([128, w], F32)
                nc.vector.scalar_tensor_tensor(Hv[:, :], Aex[:, :], invtau_ap,
                                               m1[:, :], OP.mult, OP.add)
                src = pool.tile([128, w], F32)
                nc.vector.scalar_tensor_tensor(src[:, :], ro_o, 1.0, Hv[:, :],
                                               OP.mult, OP.mult,
                                               accum_out=acc[:, t:t + 1])

                # ---- stacked TVD stencil (ro and V together) ----
                d = pool.tile([128, 2, w + 2], F32)
                nc.vector.tensor_sub(d[:, :, :], z2[:, :, 1:w + 3],
                                     z2[:, :, 0:w + 2])
                s2 = pool.tile([128, 2, w + 1], F32)
                nc.vector.tensor_sub(s2[:, :, :], z2[:, :, 2:w + 3],
                                     z2[:, :, 0:w + 1])
                x1 = pool.tile([128, 2, w + 1], F32)
                nc.scalar.activation(x1[:, :, :], s2[:, :, :], AF.Abs,
                                     scale=COEF / DTS * 0.5)
                A2 = pool.tile([128, 2, w + 2], F32)
                nc.scalar.activation(A2[:, :, :], d[:, :, :], AF.Abs,
                                     scale=COEF / DTS * 2.0)
                mA = pool.tile([128, 2, w + 1], F32)
                nc.vector.tensor_tensor(mA[:, :, :], A2[:, :, 1:w + 2],
                                        A2[:, :, 0:w + 1], OP.min)
                wi = pool.tile([128, 2, w + 1], F32)
                nc.vector.tensor_tensor(wi[:, :, :], x1[:, :, :], mA[:, :, :],
                                        OP.min)
                rp = pool.tile([128, 2, w], F32)
                nc.vector.tensor_sub(rp[:, :, :], wi[:, :, 1:w + 1],
                                     wi[:, :, 0:w])
                s1 = pool.tile([128, 2, w], F32)
                nc.vector.scalar_tensor_tensor(s1[:, :, :], d[:, :, 1:w + 1],
                                               -1.0 / DTS, rp[:, :, :],
                                               OP.mult, OP.subtract)
                fro = pool.tile([128, w], F32)
                nc.vector.tensor_sub(fro[:, :], s1[:, 0, :], src[:, :])
                fV = pool.tile([128, w], F32)
                nc.vector.tensor_add(fV[:, :], s1[:, 1, :], dVdt[:, :])
                nc.sync.dma_start(out=dout_ap[0, :, c0:c0 + w], in_=fro[:, :])
                nc.sync.dma_start(out=dout_ap[1, :, c0:c0 + w], in_=fV[:, :])

            accsum = pp.tile([128, 1], F32)
            nc.vector.tensor_reduce(accsum[:, :], acc[:, :],
                                    axis=mybir.AxisListType.X, op=OP.add)
            nc.sync.dma_start(out=accout_ap, in_=accsum[:, :])
    nc.compile()
    return nc


_NC_CACHE = {}


def _get_program(lw=LW, w=W):
    key = (lw, w)
    if key not in _NC_CACHE:
        _NC_CACHE[key] = build_program(lw, w)
    return _NC_CACHE[key]


def run_cores(ro_pad, v_pad, b_val, invtau_val, lw=LW, w=W, ncores=NCORES,
              trace=False):
    """ro_pad/v_pad: f32 arrays of length ncores*128*lw + 3 (2 left halo,
    owned, 1 right halo). Returns (out [2, ncores*128*lw], firing_partials
    [ncores,128], results_obj)."""
    from concourse.bass_utils import run_bass_kernel_spmd

    s_own = 128 * lw
    nc = _get_program(lw, w)
    scal = np.empty((128, NSCAL), np.float32)
    scal[:, 0] = b_val
    scal[:, 1] = invtau_val
    scal[:, 2] = AL2
    scal[:, 3] = GA2
    scal[:, 4] = 1.00000001

    in_maps = []
    for c in range(ncores):
        base = c * s_own
        zin = np.empty((2, 128, lw + 3), np.float32)
        for q, arr in ((0, ro_pad), (1, v_pad)):
            view = np.lib.stride_tricks.as_strided(
                arr[base:], shape=(128, lw + 3),
                strides=(lw * arr.itemsize, arr.itemsize))
            zin[q] = view
        in_maps.append({"zin": zin, "scal": scal})

    res = run_bass_kernel_spmd(nc, in_maps, list(range(ncores)), trace=trace)
    outs = np.empty((2, ncores * s_own), np.float32)
    partials = np.empty((ncores, 128), np.float32)
    for c in range(ncores):
        m = res.results[c]
        outs[0, c * s_own:(c + 1) * s_own] = m["dout"][0].reshape(-1)
        outs[1, c * s_own:(c + 1) * s_own] = m["dout"][1].reshape(-1)
        partials[c] = m["accout"].reshape(-1)
    return outs, partials, res


def _erf(x):
    return math.erf(x)


def _H_scalar(V, dVdt, invtau):
    f32 = np.float32
    V = f32(V)
    dVdt = f32(dVdt)
    delta_V = max(f32(-V), f32(-1.0))
    T = f32(delta_V * f32(K))
    T2 = f32(T * T)
    p = f32(C0) + f32(C1) * T + f32(C2) * T2 + f32(C3) * T2 * T \
        + f32(C4) * T2 * T2
    A = np.exp(p, dtype=f32)
    den = f32(_erf(float(T)) + 1.00000001)
    F = np.exp(f32(-T2 - np.log(den, dtype=f32)), dtype=f32)
    g = max(dVdt * f32(CC), f32(0.0))
    return f32(A * f32(invtau) + g * F)


def _limiter(a, b):
    return min(0.5 * abs(a + b), 2.0 * min(abs(a), abs(b)))


def kernel(t=None, y=None, gsyn=None, Isyn=None, **_ignored):
    f32 = np.float32
    y = np.asarray(y, f32)
    ro = y[:N]
    V = y[N:]
    Isyn_s = float(np.asarray(Isyn, f32).reshape(-1)[0])
    gsum = float(np.sum(np.asarray(gsyn, f32), dtype=f32))
    tau_m = Cm / (GL + gsum)
    invtau = 1.0 / tau_m
    b_val = (GL * EL + IEXT + Isyn_s) / Cm

    # padded inputs: [2 halo][N][pad zeros][1 halo]; left halo = dup of elem 0
    ro_pad = np.zeros(2 + TOT + 1, f32)
    ro_pad[0:2] = ro[0]
    ro_pad[2:2 + N] = ro
    v_pad = np.zeros(2 + TOT + 1, f32)
    v_pad[0:2] = V[0]
    v_pad[2:2 + N] = V

    outs, partials, _ = run_cores(ro_pad, v_pad, b_val, invtau)

    firing = f32(np.sum(partials, dtype=np.float64))
    dro = outs[0][:N]
    dV = outs[1][:N]
    # host fixups (4 edge elements)
    dro[0] = -ro[0] / f32(DTS) + firing
    wi_last = _limiter(float(ro[N - 1]) - float(ro[N - 2]),
                       float(ro[N - 2]) - float(ro[N - 3]))
    dVdt_last = f32(A_CONST) * V[N - 1] + f32(b_val)
    src_last = ro[N - 1] * _H_scalar(V[N - 1], dVdt_last, invtau)
    dro[N - 1] = (ro[N - 2] + f32(COEF) * f32(wi_last)) / f32(DTS) - src_last
    dV[0] = 0.0
    dV[N - 1] = dVdt_last
    return np.concatenate([dro, dV])


# revision 7
# speedup vs baseline: 1.2562x; 1.2562x over previous
"""Trainium2 Bass kernel for nn_Network_10256381903586.

Population-density LIF network RHS:
  y = [ro (N), V (N)] -> dy/dt, N = 8,000,000.

Decomposition across 8 NeuronCores (data-parallel, no collectives):
  - Each core owns a contiguous chunk of S_OWN = 2^20 grid points of both
    ro and V (total 8*2^20 >= N; tail is zero-padded).
  - Per-core inputs carry a 2-left/1-right element halo so the 4-point TVD
    stencil is uniform everywhere; global edge cells (4 elements) and the
    firing-rate feedback (a single scalar = sum(ro*H), which only affects
    output element 0) are patched on the host from per-core partial sums.
  - Layout on core: chunk viewed as [128 partitions x LW=8192] row-major
    (partition p = contiguous segment), so the stencil is a free-axis
    shift. Tiles of width W columns, each loaded with a 3-column halo.

Math notes (exact rewrites of the reference):
  - limiter(a,b) = min(0.5|a+b|, 2min(|a|,|b|))  (the reference's masked
    sequence reduces to this because its two index sets are disjoint).
  - The quartic exp argument is factored into two quadratics so the ACT
    engine's Square(scale*x+bias) evaluates most of it.
  - exp(-T^2)/(1.00000001+erf(T)) = exp(-(T^2 + ln(1.00000001+erf(T)))).
"""
import math

import numpy as np

# ---------------- problem constants ----------------
N = 8_000_000
GL = 0.1
EL = -5.0
Cm = 0.3
IEXT = 0.4
DTS = 0.5
DT = 0.1
SQ2 = math.sqrt(2.0)
SQ2PI = 0.7978845608028654
SIGMA = 0.3 / GL * math.sqrt(0.5 * GL / Cm)
COEF = 0.5 * (1.0 - DT / DTS)            # 0.4
K = 1.0 / (SIGMA * SQ2)                  # T = K * delta_V  (= 1/sqrt(3))
CC = SQ2 * K * SQ2PI                     # g = relu(CC * dVdt)
A_CONST = -GL / Cm

# quartic p(T) = C4*T^4 + ... + C0 factored: C4*(T^2+al*T+be)(T^2+ga*T+de)
C0, C1, C2, C3, C4 = 0.0061, -1.12, -0.257, -0.072, -0.0117


def _quartic_factors():
    r = np.roots([C4, C3, C2, C1, C0])
    used = [False] * 4
    quads = []
    for i in range(4):
        if used[i]:
            continue
        ri = r[i]
        if abs(ri.imag) > 1e-12:
            for j in range(i + 1, 4):
                if not used[j] and abs(r[j] - np.conj(ri)) < 1e-8:
                    used[i] = used[j] = True
                    quads.append((-(2 * ri.real), (ri * np.conj(ri)).real))
                    break
        else:
            for j in range(i + 1, 4):
                if not used[j] and abs(r[j].imag) < 1e-12:
                    used[i] = used[j] = True
                    quads.append((-(ri + r[j]).real, (ri * r[j]).real))
                    break
    (al, be), (ga, de) = quads
    return al, be, ga, de


_AL, _BE, _GA, _DE = _quartic_factors()
AL2 = _AL / 2.0
GA2 = _GA / 2.0
E1 = _BE - _AL * _AL / 4.0
E2 = _DE - _GA * _GA / 4.0

NSCAL = 5
NCORES = 8
LW = 8192                 # row length per partition
S_OWN = 128 * LW          # 2^20 owned elements per core
TOT = NCORES * S_OWN
W = 512                   # tile width (columns)


# ---------------- Bass program ----------------
def build_program(lw=LW, w=W):
    import concourse.bacc as bacc
    import concourse.mybir as mybir
    import concourse.tile as tile

    AF = mybir.ActivationFunctionType
    OP = mybir.AluOpType
    F32 = mybir.dt.float32
    nt = lw // w
    assert lw % w == 0
    wa = min(lw, 2048)                     # phase-A (erf) tile width
    nta = lw // wa

    nc = bacc.Bacc("TRN2", target_bir_lowering=False, debug=False)
    zin = nc.dram_tensor("zin", [2, 128, lw + 3], F32, kind="ExternalInput")
    scal = nc.dram_tensor("scal", [128, NSCAL], F32, kind="ExternalInput")
    dout = nc.dram_tensor("dout", [2, 128, lw], F32, kind="ExternalOutput")
    accout = nc.dram_tensor("accout", [128, 1], F32, kind="ExternalOutput")
    zin_ap, scal_ap = zin.ap(), scal.ap()
    zin_r = zin_ap.rearrange("q p c -> p q c")
    dout_r = dout.ap().rearrange("q p c -> p q c")
    accout_ap = accout.ap()

    with tile.TileContext(nc) as tc:
        with tc.tile_pool(name="main", bufs=2) as pool, \
             tc.tile_pool(name="persist", bufs=1) as pp:
            scal_sb = pp.tile([128, NSCAL], F32)
            nc.sync.dma_start(out=scal_sb[:, :], in_=scal_ap)
            negb_ap = scal_sb[:, 0:1]
            invtau_ap = scal_sb[:, 1:2]
            al2_ap = scal_sb[:, 2:3]
            ga2_ap = scal_sb[:, 3:4]
            one_ap = scal_sb[:, 4:5]
            acc = pp.tile([128, nt], F32)
            erf_full = pp.tile([128, lw], F32)

            # ---- phase A: all Erf ops (single act-table set) ----
            for t in range(nta):
                a0 = t * wa
                Vt = pool.tile([128, wa], F32)
                nc.sync.dma_start(out=Vt[:, :], in_=zin_ap[1, :, a0 + 2:a0 + 2 + wa])
                dvA = pool.tile([128, wa], F32)
                nc.vector.tensor_scalar(dvA[:, :], Vt[:, :], -1.0, -1.0,
                                        OP.mult, OP.max)
                nc.scalar.activation(erf_full[:, a0:a0 + wa], dvA[:, :],
                                     AF.Erf, scale=K)

            # ---- phase B: everything else (natural_log_exp set) ----
            for t in range(nt):
                c0 = t * w
                z2 = pool.tile([128, 2, w + 3], F32)
                nc.sync.dma_start(out=z2[:, :, :], in_=zin_r[:, :, c0:c0 + w + 3])
                Vo = z2[:, 1, 2:w + 2]
                ro_o = z2[:, 0, 2:w + 2]

                # sd[:,0]=src, sd[:,1]=-dVdt
                sd = pool.tile([128, 2, w], F32)
                nc.scalar.activation(sd[:, 1, :], Vo, AF.Identity,
                                     bias=negb_ap, scale=-A_CONST)
                delta_V = pool.tile([128, w], F32)
                nc.vector.tensor_scalar(delta_V[:, :], Vo, -1.0, -1.0,
                                        OP.mult, OP.max)
                T2 = pool.tile([128, w], F32)
                nc.scalar.activation(T2[:, :], delta_V[:, :], AF.Square, scale=K)
                Q1 = pool.tile([128, w], F32)
                nc.scalar.activation(Q1[:, :], delta_V[:, :], AF.Square,
                                     bias=al2_ap, scale=K)
                Q2 = pool.tile([128, w], F32)
                nc.scalar.activation(Q2[:, :], delta_V[:, :], AF.Square,
                                     bias=ga2_ap, scale=K)
                U2 = pool.tile([128, w], F32)
                nc.vector.tensor_scalar(U2[:, :], Q2[:, :], E2, None, OP.add)
                PT = pool.tile([128, w], F32)
                nc.vector.scalar_tensor_tensor(PT[:, :], Q1[:, :], E1, U2[:, :],
                                               OP.add, OP.mult)
                Aex = pool.tile([128, w], F32)
                nc.scalar.activation(Aex[:, :], PT[:, :], AF.Exp, scale=C4)
                lnden = pool.tile([128, w], F32)
                nc.scalar.activation(lnden[:, :], erf_full[:, c0:c0 + w],
                                     AF.Ln, bias=one_ap)
                r1 = pool.tile([128, w], F32)
                nc.vector.tensor_add(r1[:, :], T2[:, :], lnden[:, :])
                Fden = pool.tile([128, w], F32)
                nc.scalar.activation(Fden[:, :], r1[:, :], AF.Exp, scale=-1.0)
                g = pool.tile([128, w], F32)
                nc.scalar.activation(g[:, :], sd[:, 1, :], AF.Relu, scale=-CC)
                m1 = pool.tile([128, w], F32)
                nc.vector.tensor_mul(m1[:, :], g[:, :], Fden[:, :])
                Hv = pool.tile([128, w], F32)
                nc.vector.scalar_tensor_tensor(Hv[:, :], Aex[:, :], invtau_ap,
                                               m1[:, :], OP.mult, OP.add)
                nc.vector.scalar_tensor_tensor(sd[:, 0, :], ro_o, 1.0, Hv[:, :],
                                               OP.mult, OP.mult,
                                               accum_out=acc[:, t:t + 1])

                # ---- stacked TVD stencil (ro and V together) ----
                d = pool.tile([128, 2, w + 2], F32)
                nc.vector.tensor_sub(d[:, :, :], z2[:, :, 1:w + 3],
                                     z2[:, :, 0:w + 2])
                s2 = pool.tile([128, 2, w + 1], F32)
                nc.vector.tensor_sub(s2[:, :, :], z2[:, :, 2:w + 3],
                                     z2[:, :, 0:w + 1])
                x1 = pool.tile([128, 2, w + 1], F32)
                nc.scalar.activation(x1[:, :, :], s2[:, :, :], AF.Abs,
                                     scale=COEF / DTS * 0.5)
                A2 = pool.tile([128, 2, w + 2], F32)
                nc.scalar.activation(A2[:, :, :], d[:, :, :], AF.Abs,
                                     scale=COEF / DTS * 2.0)
                mA = pool.tile([128, 2, w + 1], F32)
                nc.vector.tensor_tensor(mA[:, :, :], A2[:, :, 1:w + 2],
                                        A2[:, :, 0:w + 1], OP.min)
                wi = pool.tile([128, 2, w + 1], F32)
                nc.vector.tensor_tensor(wi[:, :, :], x1[:, :, :], mA[:, :, :],
                                        OP.min)
                rp = pool.tile([128, 2, w], F32)
                nc.vector.tensor_sub(rp[:, :, :], wi[:, :, 1:w + 1],
                                     wi[:, :, 0:w])
                s1 = pool.tile([128, 2, w], F32)
                nc.vector.scalar_tensor_tensor(s1[:, :, :], d[:, :, 1:w + 1],
                                               -1.0 / DTS, rp[:, :, :],
                                               OP.mult, OP.subtract)
                f = pool.tile([128, 2, w], F32)
                nc.vector.tensor_sub(f[:, :, :], s1[:, :, :], sd[:, :, :])
                nc.sync.dma_start(out=dout_r[:, :, c0:c0 + w], in_=f[:, :, :])

            accsum = pp.tile([128, 1], F32)
            nc.vector.tensor_reduce(accsum[:, :], acc[:, :],
                                    axis=mybir.AxisListType.X, op=OP.add)
            nc.sync.dma_start(out=accout_ap, in_=accsum[:, :])
    nc.compile()
    return nc


_NC_CACHE = {}


def _get_program(lw=LW, w=W):
    key = (lw, w)
    if key not in _NC_CACHE:
        _NC_CACHE[key] = build_program(lw, w)
    return _NC_CACHE[key]


def run_cores(ro_pad, v_pad, b_val, invtau_val, lw=LW, w=W, ncores=NCORES,
              trace=False):
    """ro_pad/v_pad: f32 arrays of length ncores*128*lw + 3 (2 left halo,
    owned, 1 right halo). Returns (out [2, ncores*128*lw], firing_partials
    [ncores,128], results_obj)."""
    from concourse.bass_utils import run_bass_kernel_spmd

    s_own = 128 * lw
    nc = _get_program(lw, w)
    scal = np.empty((128, NSCAL), np.float32)
    scal[:, 0] = -b_val
    scal[:, 1] = invtau_val
    scal[:, 2] = AL2
    scal[:, 3] = GA2
    scal[:, 4] = 1.00000001

    in_maps = []
    for c in range(ncores):
        base = c * s_own
        zin = np.empty((2, 128, lw + 3), np.float32)
        for q, arr in ((0, ro_pad), (1, v_pad)):
            view = np.lib.stride_tricks.as_strided(
                arr[base:], shape=(128, lw + 3),
                strides=(lw * arr.itemsize, arr.itemsize))
            zin[q] = view
        in_maps.append({"zin": zin, "scal": scal})

    res = run_bass_kernel_spmd(nc, in_maps, list(range(ncores)), trace=trace)
    outs = np.empty((2, ncores * s_own), np.float32)
    partials = np.empty((ncores, 128), np.float32)
    for c in range(ncores):
        m = res.results[c]
        outs[0, c * s_own:(c + 1) * s_own] = m["dout"][0].reshape(-1)
        outs[1, c * s_own:(c + 1) * s_own] = m["dout"][1].reshape(-1)
        partials[c] = m["accout"].reshape(-1)
    return outs, partials, res


def _erf(x):
    return math.erf(x)


def _H_scalar(V, dVdt, invtau):
    f32 = np.float32
    V = f32(V)
    dVdt = f32(dVdt)
    delta_V = max(f32(-V), f32(-1.0))
    T = f32(delta_V * f32(K))
    T2 = f32(T * T)
    p = f32(C0) + f32(C1) * T + f32(C2) * T2 + f32(C3) * T2 * T \
        + f32(C4) * T2 * T2
    A = np.exp(p, dtype=f32)
    den = f32(_erf(float(T)) + 1.00000001)
    F = np.exp(f32(-T2 - np.log(den, dtype=f32)), dtype=f32)
    g = max(dVdt * f32(CC), f32(0.0))
    return f32(A * f32(invtau) + g * F)


def _limiter(a, b):
    return min(0.5 * abs(a + b), 2.0 * min(abs(a), abs(b)))


def kernel(t=None, y=None, gsyn=None, Isyn=None, **_ignored):
    f32 = np.float32
    y = np.asarray(y, f32)
    ro = y[:N]
    V = y[N:]
    Isyn_s = float(np.asarray(Isyn, f32).reshape(-1)[0])
    gsum = float(np.sum(np.asarray(gsyn, f32), dtype=f32))
    tau_m = Cm / (GL + gsum)
    invtau = 1.0 / tau_m
    b_val = (GL * EL + IEXT + Isyn_s) / Cm

    # padded inputs: [2 halo][N][pad zeros][1 halo]; left halo = dup of elem 0
    ro_pad = np.zeros(2 + TOT + 1, f32)
    ro_pad[0:2] = ro[0]
    ro_pad[2:2 + N] = ro
    v_pad = np.zeros(2 + TOT + 1, f32)
    v_pad[0:2] = V[0]
    v_pad[2:2 + N] = V

    outs, partials, _ = run_cores(ro_pad, v_pad, b_val, invtau)

    firing = f32(np.sum(partials, dtype=np.float64))
    dro = outs[0][:N]
    dV = outs[1][:N]
    # host fixups (4 edge elements)
    dro[0] = -ro[0] / f32(DTS) + firing
    wi_last = _limiter(float(ro[N - 1]) - float(ro[N - 2]),
                       float(ro[N - 2]) - float(ro[N - 3]))
    dVdt_last = f32(A_CONST) * V[N - 1] + f32(b_val)
    src_last = ro[N - 1] * _H_scalar(V[N - 1], dVdt_last, invtau)
    dro[N - 1] = (ro[N - 2] + f32(COEF) * f32(wi_last)) / f32(DTS) - src_last
    dV[0] = 0.0
    dV[N - 1] = dVdt_last
    return np.concatenate([dro, dV])


# revision 8
# speedup vs baseline: 1.3171x; 1.0485x over previous
"""Trainium2 Bass kernel for nn_Network_10256381903586.

Population-density LIF network RHS:
  y = [ro (N), V (N)] -> dy/dt, N = 8,000,000.

Decomposition across 8 NeuronCores (data-parallel, no collectives):
  - Each core owns a contiguous chunk of S_OWN = 2^20 grid points of both
    ro and V (total 8*2^20 >= N; tail is zero-padded).
  - Per-core inputs carry a 2-left/1-right element halo so the 4-point TVD
    stencil is uniform everywhere; global edge cells (4 elements) and the
    firing-rate feedback (a single scalar = sum(ro*H), which only affects
    output element 0) are patched on the host from per-core partial sums.
  - Layout on core: chunk viewed as [128 partitions x LW=8192] row-major
    (partition p = contiguous segment), so the stencil is a free-axis
    shift. Tiles of width W columns, each loaded with a 3-column halo.

Math notes (exact rewrites of the reference):
  - limiter(a,b) = min(0.5|a+b|, 2min(|a|,|b|))  (the reference's masked
    sequence reduces to this because its two index sets are disjoint).
  - The quartic exp argument is factored into two quadratics so the ACT
    engine's Square(scale*x+bias) evaluates most of it.
  - exp(-T^2)/(1.00000001+erf(T)) = exp(-(T^2 + ln(1.00000001+erf(T)))).
"""
import math

import numpy as np

# ---------------- problem constants ----------------
N = 8_000_000
GL = 0.1
EL = -5.0
Cm = 0.3
IEXT = 0.4
DTS = 0.5
DT = 0.1
SQ2 = math.sqrt(2.0)
SQ2PI = 0.7978845608028654
SIGMA = 0.3 / GL * math.sqrt(0.5 * GL / Cm)
COEF = 0.5 * (1.0 - DT / DTS)            # 0.4
K = 1.0 / (SIGMA * SQ2)                  # T = K * delta_V  (= 1/sqrt(3))
CC = SQ2 * K * SQ2PI                     # g = relu(CC * dVdt)
A_CONST = -GL / Cm

# quartic p(T) = C4*T^4 + ... + C0 factored: C4*(T^2+al*T+be)(T^2+ga*T+de)
C0, C1, C2, C3, C4 = 0.0061, -1.12, -0.257, -0.072, -0.0117


def _quartic_factors():
    r = np.roots([C4, C3, C2, C1, C0])
    used = [False] * 4
    quads = []
    for i in range(4):
        if used[i]:
            continue
        ri = r[i]
        if abs(ri.imag) > 1e-12:
            for j in range(i + 1, 4):
                if not used[j] and abs(r[j] - np.conj(ri)) < 1e-8:
                    used[i] = used[j] = True
                    quads.append((-(2 * ri.real), (ri * np.conj(ri)).real))
                    break
        else:
            for j in range(i + 1, 4):
                if not used[j] and abs(r[j].imag) < 1e-12:
                    used[i] = used[j] = True
                    quads.append((-(ri + r[j]).real, (ri * r[j]).real))
                    break
    (al, be), (ga, de) = quads
    return al, be, ga, de


_AL, _BE, _GA, _DE = _quartic_factors()
AL2 = _AL / 2.0
GA2 = _GA / 2.0
E1 = _BE - _AL * _AL / 4.0
E2 = _DE - _GA * _GA / 4.0

NSCAL = 6
NCORES = 8
LW = 8192                 # row length per partition
S_OWN = 128 * LW          # 2^20 owned elements per core
TOT = NCORES * S_OWN
W = 512                   # tile width (columns)


# ---------------- Bass program ----------------
def build_program(lw=LW, w=W):
    import concourse.bacc as bacc
    import concourse.mybir as mybir
    import concourse.tile as tile

    AF = mybir.ActivationFunctionType
    OP = mybir.AluOpType
    F32 = mybir.dt.float32
    nt = lw // w
    assert lw % w == 0
    wa = min(lw, 2048)                     # phase-A (erf) tile width
    nta = lw // wa

    nc = bacc.Bacc("TRN2", target_bir_lowering=False, debug=False)
    zin = nc.dram_tensor("zin", [2, 128, lw + 3], F32, kind="ExternalInput")
    scal = nc.dram_tensor("scal", [128, NSCAL], F32, kind="ExternalInput")
    dout = nc.dram_tensor("dout", [2, 128, lw], F32, kind="ExternalOutput")
    accout = nc.dram_tensor("accout", [128, 1], F32, kind="ExternalOutput")
    zin_ap, scal_ap = zin.ap(), scal.ap()
    zin_r = zin_ap.rearrange("q p c -> p q c")
    dout_r = dout.ap().rearrange("q p c -> p q c")
    accout_ap = accout.ap()

    with tile.TileContext(nc) as tc:
        with tc.tile_pool(name="main", bufs=2) as pool, \
             tc.tile_pool(name="persist", bufs=1) as pp:
            scal_sb = pp.tile([128, NSCAL], F32)
            nc.sync.dma_start(out=scal_sb[:, :], in_=scal_ap)
            negb_ap = scal_sb[:, 0:1]
            invtau_ap = scal_sb[:, 1:2]
            al2_ap = scal_sb[:, 2:3]
            ga2_ap = scal_sb[:, 3:4]
            one_ap = scal_sb[:, 4:5]
            e2_ap = scal_sb[:, 5:6]
            acc = pp.tile([128, nt], F32)
            erf_full = pp.tile([128, lw], F32)

            # ---- phase A: all Erf ops (single act-table set) ----
            for t in range(nta):
                a0 = t * wa
                Vt = pool.tile([128, wa], F32)
                nc.sync.dma_start(out=Vt[:, :], in_=zin_ap[1, :, a0 + 2:a0 + 2 + wa])
                nc.scalar.activation(erf_full[:, a0:a0 + wa], Vt[:, :],
                                     AF.Erf, scale=-K)

            # ---- phase B: everything else (natural_log_exp set) ----
            for t in range(nt):
                c0 = t * w
                z2 = pool.tile([128, 2, w + 3], F32)
                nc.sync.dma_start(out=z2[:, :, :], in_=zin_r[:, :, c0:c0 + w + 3])
                Vo = z2[:, 1, 2:w + 2]
                ro_o = z2[:, 0, 2:w + 2]

                # sd[:,0]=src, sd[:,1]=-dVdt
                sd = pool.tile([128, 2, w], F32)
                nc.scalar.activation(sd[:, 1, :], Vo, AF.Identity,
                                     bias=negb_ap, scale=-A_CONST)
                T2 = pool.tile([128, w], F32)
                nc.scalar.activation(T2[:, :], Vo, AF.Square, scale=-K)
                Q1 = pool.tile([128, w], F32)
                nc.scalar.activation(Q1[:, :], Vo, AF.Square,
                                     bias=al2_ap, scale=-K)
                U2 = pool.tile([128, w], F32)
                nc.scalar.activation(U2[:, :], Vo, AF.Square,
                                     bias=ga2_ap, scale=-K)
                nc.scalar.activation(U2[:, :], U2[:, :], AF.Identity,
                                     bias=e2_ap)
                PT = pool.tile([128, w], F32)
                nc.vector.scalar_tensor_tensor(PT[:, :], Q1[:, :], E1, U2[:, :],
                                               OP.add, OP.mult)
                Aex = pool.tile([128, w], F32)
                nc.scalar.activation(Aex[:, :], PT[:, :], AF.Exp, scale=C4)
                lnden = pool.tile([128, w], F32)
                nc.scalar.activation(lnden[:, :], erf_full[:, c0:c0 + w],
                                     AF.Ln, bias=one_ap)
                r1 = pool.tile([128, w], F32)
                nc.vector.tensor_add(r1[:, :], T2[:, :], lnden[:, :])
                Fden = pool.tile([128, w], F32)
                nc.scalar.activation(Fden[:, :], r1[:, :], AF.Exp, scale=-1.0)
                g = pool.tile([128, w], F32)
                nc.scalar.activation(g[:, :], sd[:, 1, :], AF.Relu, scale=-CC)
                m1 = pool.tile([128, w], F32)
                nc.vector.tensor_mul(m1[:, :], g[:, :], Fden[:, :])
                Hv = pool.tile([128, w], F32)
                nc.vector.scalar_tensor_tensor(Hv[:, :], Aex[:, :], invtau_ap,
                                               m1[:, :], OP.mult, OP.add)
                nc.vector.scalar_tensor_tensor(sd[:, 0, :], ro_o, 1.0, Hv[:, :],
                                               OP.mult, OP.mult,
                                               accum_out=acc[:, t:t + 1])

                # ---- stacked TVD stencil (ro and V together) ----
                d = pool.tile([128, 2, w + 2], F32)
                nc.vector.tensor_sub(d[:, :, :], z2[:, :, 1:w + 3],
                                     z2[:, :, 0:w + 2])
                s2 = pool.tile([128, 2, w + 1], F32)
                nc.vector.tensor_sub(s2[:, :, :], z2[:, :, 2:w + 3],
                                     z2[:, :, 0:w + 1])
                x1 = pool.tile([128, 2, w + 1], F32)
                nc.scalar.activation(x1[:, :, :], s2[:, :, :], AF.Abs,
                                     scale=COEF / DTS * 0.5)
                A2 = pool.tile([128, 2, w + 2], F32)
                nc.scalar.activation(A2[:, :, :], d[:, :, :], AF.Abs,
                                     scale=COEF / DTS * 2.0)
                mA = pool.tile([128, 2, w + 1], F32)
                nc.vector.tensor_tensor(mA[:, :, :], A2[:, :, 1:w + 2],
                                        A2[:, :, 0:w + 1], OP.min)
                wi = pool.tile([128, 2, w + 1], F32)
                nc.vector.tensor_tensor(wi[:, :, :], x1[:, :, :], mA[:, :, :],
                                        OP.min)
                rp = pool.tile([128, 2, w], F32)
                nc.vector.tensor_sub(rp[:, :, :], wi[:, :, 1:w + 1],
                                     wi[:, :, 0:w])
                s1 = pool.tile([128, 2, w], F32)
                nc.vector.scalar_tensor_tensor(s1[:, :, :], d[:, :, 1:w + 1],
                                               -1.0 / DTS, rp[:, :, :],
                                               OP.mult, OP.subtract)
                f = pool.tile([128, 2, w], F32)
                nc.vector.tensor_sub(f[:, :, :], s1[:, :, :], sd[:, :, :])
                nc.sync.dma_start(out=dout_r[:, :, c0:c0 + w], in_=f[:, :, :])

            accsum = pp.tile([128, 1], F32)
            nc.vector.tensor_reduce(accsum[:, :], acc[:, :],
                                    axis=mybir.AxisListType.X, op=OP.add)
            nc.sync.dma_start(out=accout_ap, in_=accsum[:, :])
    nc.compile()
    return nc


_NC_CACHE = {}


def _get_program(lw=LW, w=W):
    key = (lw, w)
    if key not in _NC_CACHE:
        _NC_CACHE[key] = build_program(lw, w)
    return _NC_CACHE[key]


def run_cores(ro_pad, v_pad, b_val, invtau_val, lw=LW, w=W, ncores=NCORES,
              trace=False):
    """ro_pad/v_pad: f32 arrays of length ncores*128*lw + 3 (2 left halo,
    owned, 1 right halo). Returns (out [2, ncores*128*lw], firing_partials
    [ncores,128], results_obj)."""
    from concourse.bass_utils import run_bass_kernel_spmd

    s_own = 128 * lw
    nc = _get_program(lw, w)
    scal = np.empty((128, NSCAL), np.float32)
    scal[:, 0] = -b_val
    scal[:, 1] = invtau_val
    scal[:, 2] = AL2
    scal[:, 3] = GA2
    scal[:, 4] = 1.00000001
    scal[:, 5] = E2

    in_maps = []
    for c in range(ncores):
        base = c * s_own
        zin = np.empty((2, 128, lw + 3), np.float32)
        for q, arr in ((0, ro_pad), (1, v_pad)):
            view = np.lib.stride_tricks.as_strided(
                arr[base:], shape=(128, lw + 3),
                strides=(lw * arr.itemsize, arr.itemsize))
            zin[q] = view
        in_maps.append({"zin": zin, "scal": scal})

    res = run_bass_kernel_spmd(nc, in_maps, list(range(ncores)), trace=trace)
    outs = np.empty((2, ncores * s_own), np.float32)
    partials = np.empty((ncores, 128), np.float32)
    for c in range(ncores):
        m = res.results[c]
        outs[0, c * s_own:(c + 1) * s_own] = m["dout"][0].reshape(-1)
        outs[1, c * s_own:(c + 1) * s_own] = m["dout"][1].reshape(-1)
        partials[c] = m["accout"].reshape(-1)
    return outs, partials, res


def _erf(x):
    return math.erf(x)


def _H_scalar(V, dVdt, invtau):
    f32 = np.float32
    V = f32(V)
    dVdt = f32(dVdt)
    delta_V = max(f32(-V), f32(-1.0))
    T = f32(delta_V * f32(K))
    T2 = f32(T * T)
    p = f32(C0) + f32(C1) * T + f32(C2) * T2 + f32(C3) * T2 * T \
        + f32(C4) * T2 * T2
    A = np.exp(p, dtype=f32)
    den = f32(_erf(float(T)) + 1.00000001)
    F = np.exp(f32(-T2 - np.log(den, dtype=f32)), dtype=f32)
    g = max(dVdt * f32(CC), f32(0.0))
    return f32(A * f32(invtau) + g * F)


def _limiter(a, b):
    return min(0.5 * abs(a + b), 2.0 * min(abs(a), abs(b)))


def kernel(t=None, y=None, gsyn=None, Isyn=None, **_ignored):
    f32 = np.float32
    y = np.asarray(y, f32)
    ro = y[:N]
    V = y[N:]
    Isyn_s = float(np.asarray(Isyn, f32).reshape(-1)[0])
    gsum = float(np.sum(np.asarray(gsyn, f32), dtype=f32))
    tau_m = Cm / (GL + gsum)
    invtau = 1.0 / tau_m
    b_val = (GL * EL + IEXT + Isyn_s) / Cm

    # padded inputs: [2 halo][N][pad zeros][1 halo]; left halo = dup of elem 0
    ro_pad = np.zeros(2 + TOT + 1, f32)
    ro_pad[0:2] = ro[0]
    ro_pad[2:2 + N] = ro
    v_pad = np.zeros(2 + TOT + 1, f32)
    v_pad[0:2] = V[0]
    v_pad[2:2 + N] = V

    outs, partials, _ = run_cores(ro_pad, v_pad, b_val, invtau)

    firing = f32(np.sum(partials, dtype=np.float64))
    dro = outs[0][:N]
    dV = outs[1][:N]
    # host fixups (4 edge elements)
    dro[0] = -ro[0] / f32(DTS) + firing
    wi_last = _limiter(float(ro[N - 1]) - float(ro[N - 2]),
                       float(ro[N - 2]) - float(ro[N - 3]))
    dVdt_last = f32(A_CONST) * V[N - 1] + f32(b_val)
    src_last = ro[N - 1] * _H_scalar(V[N - 1], dVdt_last, invtau)
    dro[N - 1] = (ro[N - 2] + f32(COEF) * f32(wi_last)) / f32(DTS) - src_last
    dV[0] = 0.0
    dV[N - 1] = dVdt_last
    return np.concatenate([dro, dV])


# revision 9
# speedup vs baseline: 1.3618x; 1.0340x over previous
"""Trainium2 Bass kernel for nn_Network_10256381903586.

Population-density LIF network RHS:
  y = [ro (N), V (N)] -> dy/dt, N = 8,000,000.

Decomposition across 8 NeuronCores (data-parallel, no collectives):
  - Each core owns a contiguous chunk of S_OWN = 2^20 grid points of both
    ro and V (total 8*2^20 >= N; tail is zero-padded).
  - Per-core inputs carry a 2-left/1-right element halo so the 4-point TVD
    stencil is uniform everywhere; global edge cells (4 elements) and the
    firing-rate feedback (a single scalar = sum(ro*H), which only affects
    output element 0) are patched on the host from per-core partial sums.
  - Layout on core: chunk viewed as [128 partitions x LW=8192] row-major
    (partition p = contiguous segment), so the stencil is a free-axis
    shift. Tiles of width W columns, each loaded with a 3-column halo.

Math notes (exact rewrites of the reference):
  - limiter(a,b) = min(0.5|a+b|, 2min(|a|,|b|))  (the reference's masked
    sequence reduces to this because its two index sets are disjoint).
  - The quartic exp argument is factored into two quadratics so the ACT
    engine's Square(scale*x+bias) evaluates most of it.
  - exp(-T^2)/(1.00000001+erf(T)) = exp(-(T^2 + ln(1.00000001+erf(T)))).
"""
import math

import numpy as np

# ---------------- problem constants ----------------
N = 8_000_000
GL = 0.1
EL = -5.0
Cm = 0.3
IEXT = 0.4
DTS = 0.5
DT = 0.1
SQ2 = math.sqrt(2.0)
SQ2PI = 0.7978845608028654
SIGMA = 0.3 / GL * math.sqrt(0.5 * GL / Cm)
COEF = 0.5 * (1.0 - DT / DTS)            # 0.4
K = 1.0 / (SIGMA * SQ2)                  # T = K * delta_V  (= 1/sqrt(3))
CC = SQ2 * K * SQ2PI                     # g = relu(CC * dVdt)
A_CONST = -GL / Cm

# quartic p(T) = C4*T^4 + ... + C0 factored: C4*(T^2+al*T+be)(T^2+ga*T+de)
C0, C1, C2, C3, C4 = 0.0061, -1.12, -0.257, -0.072, -0.0117


def _quartic_factors():
    r = np.roots([C4, C3, C2, C1, C0])
    used = [False] * 4
    quads = []
    for i in range(4):
        if used[i]:
            continue
        ri = r[i]
        if abs(ri.imag) > 1e-12:
            for j in range(i + 1, 4):
                if not used[j] and abs(r[j] - np.conj(ri)) < 1e-8:
                    used[i] = used[j] = True
                    quads.append((-(2 * ri.real), (ri * np.conj(ri)).real))
                    break
        else:
            for j in range(i + 1, 4):
                if not used[j] and abs(r[j].imag) < 1e-12:
                    used[i] = used[j] = True
                    quads.append((-(ri + r[j]).real, (ri * r[j]).real))
                    break
    (al, be), (ga, de) = quads
    return al, be, ga, de


_AL, _BE, _GA, _DE = _quartic_factors()
AL2 = _AL / 2.0
GA2 = _GA / 2.0
E1 = _BE - _AL * _AL / 4.0
E2 = _DE - _GA * _GA / 4.0

NSCAL = 6
NCORES = 8
LW = 8192                 # row length per partition
S_OWN = 128 * LW          # 2^20 owned elements per core
TOT = NCORES * S_OWN
W = 512                   # tile width (columns)


# ---------------- Bass program ----------------
def build_program(lw=LW, w=W):
    import concourse.bacc as bacc
    import concourse.mybir as mybir
    import concourse.tile as tile
    from concourse.tile import add_dep_helper

    AF = mybir.ActivationFunctionType
    OP = mybir.AluOpType
    F32 = mybir.dt.float32
    nt = lw // w
    assert lw % w == 0
    wa = min(lw, 2048)                     # phase-A (erf) tile width
    nta = lw // wa

    nc = bacc.Bacc("TRN2", target_bir_lowering=False, debug=False)
    zin = nc.dram_tensor("zin", [2, 128, lw + 3], F32, kind="ExternalInput")
    scal = nc.dram_tensor("scal", [128, NSCAL], F32, kind="ExternalInput")
    dout = nc.dram_tensor("dout", [2, 128, lw], F32, kind="ExternalOutput")
    accout = nc.dram_tensor("accout", [128, 1], F32, kind="ExternalOutput")
    zin_ap, scal_ap = zin.ap(), scal.ap()
    zin_r = zin_ap.rearrange("q p c -> p q c")
    dout_r = dout.ap().rearrange("q p c -> p q c")
    accout_ap = accout.ap()

    with tile.TileContext(nc) as tc:
        with tc.tile_pool(name="main", bufs=2) as pool, \
             tc.tile_pool(name="persist", bufs=1) as pp:
            scal_sb = pp.tile([128, NSCAL], F32)
            nc.sync.dma_start(out=scal_sb[:, :], in_=scal_ap)
            negb_ap = scal_sb[:, 0:1]
            invtau_ap = scal_sb[:, 1:2]
            al2_ap = scal_sb[:, 2:3]
            ga2_ap = scal_sb[:, 3:4]
            one_ap = scal_sb[:, 4:5]
            e2_ap = scal_sb[:, 5:6]
            acc = pp.tile([128, nt], F32)
            erf_full = pp.tile([128, lw], F32)

            # ---- phase A: all Erf ops (single act-table set) ----
            erf_insts = []
            for t in range(nta):
                a0 = t * wa
                Vt = pool.tile([128, wa], F32)
                nc.sync.dma_start(out=Vt[:, :], in_=zin_ap[1, :, a0 + 2:a0 + 2 + wa])
                bi = nc.scalar.activation(erf_full[:, a0:a0 + wa], Vt[:, :],
                                          AF.Erf, scale=-K)
                erf_insts.append(bi.ins)

            # ---- phase B1: all Ln ops, in place over erf_full ----
            ln_insts = []
            for t in range(nta):
                a0 = t * wa
                bi = nc.scalar.activation(erf_full[:, a0:a0 + wa],
                                          erf_full[:, a0:a0 + wa],
                                          AF.Ln, bias=one_ap)
                # keep every Ln after every Erf in the ACT stream
                add_dep_helper(bi.ins, erf_insts[-1], sync=False,
                               reason="act-table phase order: ln after erf")
                ln_insts.append(bi.ins)

            # ---- phase B: everything else (natural_log_exp set) ----
            for t in range(nt):
                c0 = t * w
                z2 = pool.tile([128, 2, w + 3], F32)
                nc.sync.dma_start(out=z2[:, :, :], in_=zin_r[:, :, c0:c0 + w + 3])
                Vo = z2[:, 1, 2:w + 2]
                ro_o = z2[:, 0, 2:w + 2]

                # sd[:,0]=src, sd[:,1]=-dVdt
                sd = pool.tile([128, 2, w], F32)
                nc.scalar.activation(sd[:, 1, :], Vo, AF.Identity,
                                     bias=negb_ap, scale=-A_CONST)
                T2 = pool.tile([128, w], F32)
                nc.scalar.activation(T2[:, :], Vo, AF.Square, scale=-K)
                Q1 = pool.tile([128, w], F32)
                nc.scalar.activation(Q1[:, :], Vo, AF.Square,
                                     bias=al2_ap, scale=-K)
                U2 = pool.tile([128, w], F32)
                nc.scalar.activation(U2[:, :], Vo, AF.Square,
                                     bias=ga2_ap, scale=-K)
                nc.scalar.activation(U2[:, :], U2[:, :], AF.Identity,
                                     bias=e2_ap)
                PT = pool.tile([128, w], F32)
                nc.vector.scalar_tensor_tensor(PT[:, :], Q1[:, :], E1, U2[:, :],
                                               OP.add, OP.mult)
                Aex = pool.tile([128, w], F32)
                bi = nc.scalar.activation(Aex[:, :], PT[:, :], AF.Exp, scale=C4)
                add_dep_helper(bi.ins, ln_insts[-1], sync=False,
                               reason="act-table phase order: exp after ln")
                r1 = pool.tile([128, w], F32)
                nc.vector.tensor_add(r1[:, :], T2[:, :], erf_full[:, c0:c0 + w])
                Fden = pool.tile([128, w], F32)
                bi = nc.scalar.activation(Fden[:, :], r1[:, :], AF.Exp, scale=-1.0)
                add_dep_helper(bi.ins, ln_insts[-1], sync=False,
                               reason="act-table phase order: exp after ln")
                g = pool.tile([128, w], F32)
                nc.scalar.activation(g[:, :], sd[:, 1, :], AF.Relu, scale=-CC)
                m1 = pool.tile([128, w], F32)
                nc.vector.tensor_mul(m1[:, :], g[:, :], Fden[:, :])
                Hv = pool.tile([128, w], F32)
                nc.vector.scalar_tensor_tensor(Hv[:, :], Aex[:, :], invtau_ap,
                                               m1[:, :], OP.mult, OP.add)
                nc.vector.scalar_tensor_tensor(sd[:, 0, :], ro_o, 1.0, Hv[:, :],
                                               OP.mult, OP.mult,
                                               accum_out=acc[:, t:t + 1])

                # ---- stacked TVD stencil (ro and V together) ----
                d = pool.tile([128, 2, w + 2], F32)
                nc.vector.tensor_sub(d[:, :, :], z2[:, :, 1:w + 3],
                                     z2[:, :, 0:w + 2])
                s2 = pool.tile([128, 2, w + 1], F32)
                nc.vector.tensor_sub(s2[:, :, :], z2[:, :, 2:w + 3],
                                     z2[:, :, 0:w + 1])
                x1 = pool.tile([128, 2, w + 1], F32)
                nc.scalar.activation(x1[:, :, :], s2[:, :, :], AF.Abs,
                                     scale=COEF / DTS * 0.5)
                A2 = pool.tile([128, 2, w + 2], F32)
                nc.scalar.activation(A2[:, :, :], d[:, :, :], AF.Abs,
                                     scale=COEF / DTS * 2.0)
                mA = pool.tile([128, 2, w + 1], F32)
                nc.vector.tensor_tensor(mA[:, :, :], A2[:, :, 1:w + 2],
                                        A2[:, :, 0:w + 1], OP.min)
                wi = pool.tile([128, 2, w + 1], F32)
                nc.vector.tensor_tensor(wi[:, :, :], x1[:, :, :], mA[:, :, :],
                                        OP.min)
                rp = pool.tile([128, 2, w], F32)
                nc.vector.tensor_sub(rp[:, :, :], wi[:, :, 1:w + 1],
                                     wi[:, :, 0:w])
                s1 = pool.tile([128, 2, w], F32)
                nc.vector.scalar_tensor_tensor(s1[:, :, :], d[:, :, 1:w + 1],
                                               -1.0 / DTS, rp[:, :, :],
                                               OP.mult, OP.subtract)
                f = pool.tile([128, 2, w], F32)
                nc.vector.tensor_sub(f[:, :, :], s1[:, :, :], sd[:, :, :])
                nc.sync.dma_start(out=dout_r[:, :, c0:c0 + w], in_=f[:, :, :])

            accsum = pp.tile([128, 1], F32)
            nc.vector.tensor_reduce(accsum[:, :], acc[:, :],
                                    axis=mybir.AxisListType.X, op=OP.add)
            nc.sync.dma_start(out=accout_ap, in_=accsum[:, :])
    nc.compile()
    return nc


_NC_CACHE = {}


def _get_program(lw=LW, w=W):
    key = (lw, w)
    if key not in _NC_CACHE:
        _NC_CACHE[key] = build_program(lw, w)
    return _NC_CACHE[key]


def run_cores(ro_pad, v_pad, b_val, invtau_val, lw=LW, w=W, ncores=NCORES,
              trace=False):
    """ro_pad/v_pad: f32 arrays of length ncores*128*lw + 3 (2 left halo,
    owned, 1 right halo). Returns (out [2, ncores*128*lw], firing_partials
    [ncores,128], results_obj)."""
    from concourse.bass_utils import run_bass_kernel_spmd

    s_own = 128 * lw
    nc = _get_program(lw, w)
    scal = np.empty((128, NSCAL), np.float32)
    scal[:, 0] = -b_val
    scal[:, 1] = invtau_val
    scal[:, 2] = AL2
    scal[:, 3] = GA2
    scal[:, 4] = 1.00000001
    scal[:, 5] = E2

    in_maps = []
    for c in range(ncores):
        base = c * s_own
        zin = np.empty((2, 128, lw + 3), np.float32)
        for q, arr in ((0, ro_pad), (1, v_pad)):
            view = np.lib.stride_tricks.as_strided(
                arr[base:], shape=(128, lw + 3),
                strides=(lw * arr.itemsize, arr.itemsize))
            zin[q] = view
        in_maps.append({"zin": zin, "scal": scal})

    res = run_bass_kernel_spmd(nc, in_maps, list(range(ncores)), trace=trace)
    outs = np.empty((2, ncores * s_own), np.float32)
    partials = np.empty((ncores, 128), np.float32)
    for c in range(ncores):
        m = res.results[c]
        outs[0, c * s_own:(c + 1) * s_own] = m["dout"][0].reshape(-1)
        outs[1, c * s_own:(c + 1) * s_own] = m["dout"][1].reshape(-1)
        partials[c] = m["accout"].reshape(-1)
    return outs, partials, res


def _erf(x):
    return math.erf(x)


def _H_scalar(V, dVdt, invtau):
    f32 = np.float32
    V = f32(V)
    dVdt = f32(dVdt)
    delta_V = max(f32(-V), f32(-1.0))
    T = f32(delta_V * f32(K))
    T2 = f32(T * T)
    p = f32(C0) + f32(C1) * T + f32(C2) * T2 + f32(C3) * T2 * T \
        + f32(C4) * T2 * T2
    A = np.exp(p, dtype=f32)
    den = f32(_erf(float(T)) + 1.00000001)
    F = np.exp(f32(-T2 - np.log(den, dtype=f32)), dtype=f32)
    g = max(dVdt * f32(CC), f32(0.0))
    return f32(A * f32(invtau) + g * F)


def _limiter(a, b):
    return min(0.5 * abs(a + b), 2.0 * min(abs(a), abs(b)))


def kernel(t=None, y=None, gsyn=None, Isyn=None, **_ignored):
    f32 = np.float32
    y = np.asarray(y, f32)
    ro = y[:N]
    V = y[N:]
    Isyn_s = float(np.asarray(Isyn, f32).reshape(-1)[0])
    gsum = float(np.sum(np.asarray(gsyn, f32), dtype=f32))
    tau_m = Cm / (GL + gsum)
    invtau = 1.0 / tau_m
    b_val = (GL * EL + IEXT + Isyn_s) / Cm

    # padded inputs: [2 halo][N][pad zeros][1 halo]; left halo = dup of elem 0
    ro_pad = np.zeros(2 + TOT + 1, f32)
    ro_pad[0:2] = ro[0]
    ro_pad[2:2 + N] = ro
    v_pad = np.zeros(2 + TOT + 1, f32)
    v_pad[0:2] = V[0]
    v_pad[2:2 + N] = V

    outs, partials, _ = run_cores(ro_pad, v_pad, b_val, invtau)

    firing = f32(np.sum(partials, dtype=np.float64))
    dro = outs[0][:N]
    dV = outs[1][:N]
    # host fixups (4 edge elements)
    dro[0] = -ro[0] / f32(DTS) + firing
    wi_last = _limiter(float(ro[N - 1]) - float(ro[N - 2]),
                       float(ro[N - 2]) - float(ro[N - 3]))
    dVdt_last = f32(A_CONST) * V[N - 1] + f32(b_val)
    src_last = ro[N - 1] * _H_scalar(V[N - 1], dVdt_last, invtau)
    dro[N - 1] = (ro[N - 2] + f32(COEF) * f32(wi_last)) / f32(DTS) - src_last
    dV[0] = 0.0
    dV[N - 1] = dVdt_last
    return np.concatenate([dro, dV])


# revision 10
# speedup vs baseline: 1.3631x; 1.0009x over previous
"""Trainium2 Bass kernel for nn_Network_10256381903586.

Population-density LIF network RHS:
  y = [ro (N), V (N)] -> dy/dt, N = 8,000,000.

Decomposition across 8 NeuronCores (data-parallel, no collectives):
  - Each core owns a contiguous chunk of S_OWN = 2^20 grid points of both
    ro and V (total 8*2^20 >= N; tail is zero-padded).
  - Per-core inputs carry a 2-left/1-right element halo so the 4-point TVD
    stencil is uniform everywhere; global edge cells (4 elements) and the
    firing-rate feedback (a single scalar = sum(ro*H), which only affects
    output element 0) are patched on the host from per-core partial sums.
  - Layout on core: chunk viewed as [128 partitions x LW=8192] row-major
    (partition p = contiguous segment), so the stencil is a free-axis
    shift. Tiles of width W columns, each loaded with a 3-column halo.

Math notes (exact rewrites of the reference):
  - limiter(a,b) = min(0.5|a+b|, 2min(|a|,|b|))  (the reference's masked
    sequence reduces to this because its two index sets are disjoint).
  - The quartic exp argument is factored into two quadratics so the ACT
    engine's Square(scale*x+bias) evaluates most of it.
  - exp(-T^2)/(1.00000001+erf(T)) = exp(-(T^2 + ln(1.00000001+erf(T)))).
"""
import math

import numpy as np

# ---------------- problem constants ----------------
N = 8_000_000
GL = 0.1
EL = -5.0
Cm = 0.3
IEXT = 0.4
DTS = 0.5
DT = 0.1
SQ2 = math.sqrt(2.0)
SQ2PI = 0.7978845608028654
SIGMA = 0.3 / GL * math.sqrt(0.5 * GL / Cm)
COEF = 0.5 * (1.0 - DT / DTS)            # 0.4
K = 1.0 / (SIGMA * SQ2)                  # T = K * delta_V  (= 1/sqrt(3))
CC = SQ2 * K * SQ2PI                     # g = relu(CC * dVdt)
A_CONST = -GL / Cm

# quartic p(T) = C4*T^4 + ... + C0 factored: C4*(T^2+al*T+be)(T^2+ga*T+de)
C0, C1, C2, C3, C4 = 0.0061, -1.12, -0.257, -0.072, -0.0117


def _quartic_factors():
    r = np.roots([C4, C3, C2, C1, C0])
    used = [False] * 4
    quads = []
    for i in range(4):
        if used[i]:
            continue
        ri = r[i]
        if abs(ri.imag) > 1e-12:
            for j in range(i + 1, 4):
                if not used[j] and abs(r[j] - np.conj(ri)) < 1e-8:
                    used[i] = used[j] = True
                    quads.append((-(2 * ri.real), (ri * np.conj(ri)).real))
                    break
        else:
            for j in range(i + 1, 4):
                if not used[j] and abs(r[j].imag) < 1e-12:
                    used[i] = used[j] = True
                    quads.append((-(ri + r[j]).real, (ri * r[j]).real))
                    break
    (al, be), (ga, de) = quads
    return al, be, ga, de


_AL, _BE, _GA, _DE = _quartic_factors()
AL2 = _AL / 2.0
GA2 = _GA / 2.0
E1 = _BE - _AL * _AL / 4.0
E2 = _DE - _GA * _GA / 4.0

NSCAL = 6
NCORES = 8
LW = 8192                 # row length per partition
S_OWN = 128 * LW          # 2^20 owned elements per core
TOT = NCORES * S_OWN
W = 1024                  # tile width (columns)


# ---------------- Bass program ----------------
def build_program(lw=LW, w=W):
    import concourse.bacc as bacc
    import concourse.mybir as mybir
    import concourse.tile as tile
    from concourse.tile import add_dep_helper

    AF = mybir.ActivationFunctionType
    OP = mybir.AluOpType
    F32 = mybir.dt.float32
    nt = lw // w
    assert lw % w == 0
    wa = min(lw, 2048)                     # phase-A (erf/ln) tile width
    nta = lw // wa

    nc = bacc.Bacc("TRN2", target_bir_lowering=False, debug=False)
    zin = nc.dram_tensor("zin", [2, 128, lw + 3], F32, kind="ExternalInput")
    scal = nc.dram_tensor("scal", [128, NSCAL], F32, kind="ExternalInput")
    dout = nc.dram_tensor("dout", [2, 128, lw], F32, kind="ExternalOutput")
    accout = nc.dram_tensor("accout", [128, 1], F32, kind="ExternalOutput")
    zin_ap, scal_ap = zin.ap(), scal.ap()
    zin_r = zin_ap.rearrange("q p c -> p q c")
    dout_r = dout.ap().rearrange("q p c -> p q c")
    accout_ap = accout.ap()

    with tile.TileContext(nc) as tc:
        with tc.tile_pool(name="io", bufs=2) as pio, \
             tc.tile_pool(name="tmp1", bufs=1) as p1, \
             tc.tile_pool(name="tmp2", bufs=2) as p2, \
             tc.tile_pool(name="persist", bufs=1) as pp:
            scal_sb = pp.tile([128, NSCAL], F32)
            nc.sync.dma_start(out=scal_sb[:, :], in_=scal_ap)
            negb_ap = scal_sb[:, 0:1]
            invtau_ap = scal_sb[:, 1:2]
            al2_ap = scal_sb[:, 2:3]
            ga2_ap = scal_sb[:, 3:4]
            one_ap = scal_sb[:, 4:5]
            e2_ap = scal_sb[:, 5:6]
            acc = pp.tile([128, nt], F32)
            erf_full = pp.tile([128, lw], F32)

            # ---- phase A: all Erf ops (single act-table set) ----
            erf_insts = []
            for t in range(nta):
                a0 = t * wa
                Vt = pio.tile([128, wa], F32)
                nc.sync.dma_start(out=Vt[:, :], in_=zin_ap[1, :, a0 + 2:a0 + 2 + wa])
                bi = nc.scalar.activation(erf_full[:, a0:a0 + wa], Vt[:, :],
                                          AF.Erf, scale=-K)
                erf_insts.append(bi.ins)

            # ---- phase B1: all Ln ops, in place over erf_full ----
            ln_insts = []
            for t in range(nta):
                a0 = t * wa
                bi = nc.scalar.activation(erf_full[:, a0:a0 + wa],
                                          erf_full[:, a0:a0 + wa],
                                          AF.Ln, bias=one_ap)
                add_dep_helper(bi.ins, erf_insts[-1], sync=False,
                               reason="act-table phase order: ln after erf")
                ln_insts.append(bi.ins)

            # ---- phase B2: everything else (exp set only) ----
            for t in range(nt):
                c0 = t * w
                z2 = pio.tile([128, 2, w + 3], F32)
                nc.sync.dma_start(out=z2[:, :, :], in_=zin_r[:, :, c0:c0 + w + 3])
                Vo = z2[:, 1, 2:w + 2]
                ro_o = z2[:, 0, 2:w + 2]

                # sd[:,0]=src, sd[:,1]=-dVdt
                sd = p2.tile([128, 2, w], F32)
                nc.scalar.activation(sd[:, 1, :], Vo, AF.Identity,
                                     bias=negb_ap, scale=-A_CONST)
                T2 = p2.tile([128, w], F32)
                nc.scalar.activation(T2[:, :], Vo, AF.Square, scale=-K)
                Q1 = p2.tile([128, w], F32)
                nc.scalar.activation(Q1[:, :], Vo, AF.Square,
                                     bias=al2_ap, scale=-K)
                U2 = p2.tile([128, w], F32)
                nc.scalar.activation(U2[:, :], Vo, AF.Square,
                                     bias=ga2_ap, scale=-K)
                nc.scalar.activation(U2[:, :], U2[:, :], AF.Identity,
                                     bias=e2_ap)
                PT = Q1
                nc.vector.scalar_tensor_tensor(PT[:, :], Q1[:, :], E1, U2[:, :],
                                               OP.add, OP.mult)
                Aex = p2.tile([128, w], F32)
                bi = nc.scalar.activation(Aex[:, :], PT[:, :], AF.Exp, scale=C4)
                add_dep_helper(bi.ins, ln_insts[-1], sync=False,
                               reason="act-table phase order: exp after ln")
                r1 = T2
                nc.vector.tensor_add(r1[:, :], T2[:, :], erf_full[:, c0:c0 + w])
                Fden = p2.tile([128, w], F32)
                bi = nc.scalar.activation(Fden[:, :], r1[:, :], AF.Exp, scale=-1.0)
                add_dep_helper(bi.ins, ln_insts[-1], sync=False,
                               reason="act-table phase order: exp after ln")
                g = p2.tile([128, w], F32)
                nc.scalar.activation(g[:, :], sd[:, 1, :], AF.Relu, scale=-CC)
                m1 = g
                nc.vector.tensor_mul(m1[:, :], g[:, :], Fden[:, :])
                Hv = Aex
                nc.vector.scalar_tensor_tensor(Hv[:, :], Aex[:, :], invtau_ap,
                                               m1[:, :], OP.mult, OP.add)
                nc.vector.scalar_tensor_tensor(sd[:, 0, :], ro_o, 1.0, Hv[:, :],
                                               OP.mult, OP.mult,
                                               accum_out=acc[:, t:t + 1])

                # ---- stacked TVD stencil (ro and V together) ----
                d = p1.tile([128, 2, w + 2], F32)
                nc.vector.tensor_sub(d[:, :, :], z2[:, :, 1:w + 3],
                                     z2[:, :, 0:w + 2])
                s2 = p1.tile([128, 2, w + 1], F32)
                nc.vector.tensor_sub(s2[:, :, :], z2[:, :, 2:w + 3],
                                     z2[:, :, 0:w + 1])
                x1 = s2
                nc.scalar.activation(x1[:, :, :], s2[:, :, :], AF.Abs,
                                     scale=COEF / DTS * 0.5)
                A2 = p1.tile([128, 2, w + 2], F32)
                nc.scalar.activation(A2[:, :, :], d[:, :, :], AF.Abs,
                                     scale=COEF / DTS * 2.0)
                mA = p1.tile([128, 2, w + 1], F32)
                nc.vector.tensor_tensor(mA[:, :, :], A2[:, :, 1:w + 2],
                                        A2[:, :, 0:w + 1], OP.min)
                wi = x1
                nc.vector.tensor_tensor(wi[:, :, :], x1[:, :, :], mA[:, :, :],
                                        OP.min)
                rp = A2[:, :, 0:w]
                nc.vector.tensor_sub(rp[:, :, :], wi[:, :, 1:w + 1],
                                     wi[:, :, 0:w])
                s1 = p1.tile([128, 2, w], F32)
                nc.vector.scalar_tensor_tensor(s1[:, :, :], d[:, :, 1:w + 1],
                                               -1.0 / DTS, rp[:, :, :],
                                               OP.mult, OP.subtract)
                f = s1
                nc.vector.tensor_sub(f[:, :, :], s1[:, :, :], sd[:, :, :])
                nc.sync.dma_start(out=dout_r[:, :, c0:c0 + w], in_=f[:, :, :])

            accsum = pp.tile([128, 1], F32)
            nc.vector.tensor_reduce(accsum[:, :], acc[:, :],
                                    axis=mybir.AxisListType.X, op=OP.add)
            nc.sync.dma_start(out=accout_ap, in_=accsum[:, :])
    nc.compile()
    return nc


_NC_CACHE = {}


def _get_program(lw=LW, w=W):
    key = (lw, w)
    if key not in _NC_CACHE:
        _NC_CACHE[key] = build_program(lw, w)
    return _NC_CACHE[key]


def run_cores(ro_pad, v_pad, b_val, invtau_val, lw=LW, w=W, ncores=NCORES,
              trace=False):
    """ro_pad/v_pad: f32 arrays of length ncores*128*lw + 3 (2 left halo,
    owned, 1 right halo). Returns (out [2, ncores*128*lw], firing_partials
    [ncores,128], results_obj)."""
    from concourse.bass_utils import run_bass_kernel_spmd

    s_own = 128 * lw
    nc = _get_program(lw, w)
    scal = np.empty((128, NSCAL), np.float32)
    scal[:, 0] = -b_val
    scal[:, 1] = invtau_val
    scal[:, 2] = AL2
    scal[:, 3] = GA2
    scal[:, 4] = 1.00000001
    scal[:, 5] = E2

    in_maps = []
    for c in range(ncores):
        base = c * s_own
        zin = np.empty((2, 128, lw + 3), np.float32)
        for q, arr in ((0, ro_pad), (1, v_pad)):
            view = np.lib.stride_tricks.as_strided(
                arr[base:], shape=(128, lw + 3),
                strides=(lw * arr.itemsize, arr.itemsize))
            zin[q] = view
        in_maps.append({"zin": zin, "scal": scal})

    res = run_bass_kernel_spmd(nc, in_maps, list(range(ncores)), trace=trace)
    outs = np.empty((2, ncores * s_own), np.float32)
    partials = np.empty((ncores, 128), np.float32)
    for c in range(ncores):
        m = res.results[c]
        outs[0, c * s_own:(c + 1) * s_own] = m["dout"][0].reshape(-1)
        outs[1, c * s_own:(c + 1) * s_own] = m["dout"][1].reshape(-1)
        partials[c] = m["accout"].reshape(-1)
    return outs, partials, res


def _erf(x):
    return math.erf(x)


def _H_scalar(V, dVdt, invtau):
    f32 = np.float32
    V = f32(V)
    dVdt = f32(dVdt)
    delta_V = max(f32(-V), f32(-1.0))
    T = f32(delta_V * f32(K))
    T2 = f32(T * T)
    p = f32(C0) + f32(C1) * T + f32(C2) * T2 + f32(C3) * T2 * T \
        + f32(C4) * T2 * T2
    A = np.exp(p, dtype=f32)
    den = f32(_erf(float(T)) + 1.00000001)
    F = np.exp(f32(-T2 - np.log(den, dtype=f32)), dtype=f32)
    g = max(dVdt * f32(CC), f32(0.0))
    return f32(A * f32(invtau) + g * F)


def _limiter(a, b):
    return min(0.5 * abs(a + b), 2.0 * min(abs(a), abs(b)))


def kernel(t=None, y=None, gsyn=None, Isyn=None, **_ignored):
    f32 = np.float32
    y = np.asarray(y, f32)
    ro = y[:N]
    V = y[N:]
    Isyn_s = float(np.asarray(Isyn, f32).reshape(-1)[0])
    gsum = float(np.sum(np.asarray(gsyn, f32), dtype=f32))
    tau_m = Cm / (GL + gsum)
    invtau = 1.0 / tau_m
    b_val = (GL * EL + IEXT + Isyn_s) / Cm

    # padded inputs: [2 halo][N][pad zeros][1 halo]; left halo = dup of elem 0
    ro_pad = np.zeros(2 + TOT + 1, f32)
    ro_pad[0:2] = ro[0]
    ro_pad[2:2 + N] = ro
    v_pad = np.zeros(2 + TOT + 1, f32)
    v_pad[0:2] = V[0]
    v_pad[2:2 + N] = V

    outs, partials, _ = run_cores(ro_pad, v_pad, b_val, invtau)

    firing = f32(np.sum(partials, dtype=np.float64))
    dro = outs[0][:N]
    dV = outs[1][:N]
    # host fixups (4 edge elements)
    dro[0] = -ro[0] / f32(DTS) + firing
    wi_last = _limiter(float(ro[N - 1]) - float(ro[N - 2]),
                       float(ro[N - 2]) - float(ro[N - 3]))
    dVdt_last = f32(A_CONST) * V[N - 1] + f32(b_val)
    src_last = ro[N - 1] * _H_scalar(V[N - 1], dVdt_last, invtau)
    dro[N - 1] = (ro[N - 2] + f32(COEF) * f32(wi_last)) / f32(DTS) - src_last
    dV[0] = 0.0
    dV[N - 1] = dVdt_last
    return np.concatenate([dro, dV])


# revision 11
# speedup vs baseline: 1.3927x; 1.0218x over previous
"""Trainium2 Bass kernel for nn_Network_10256381903586.

Population-density LIF network RHS:
  y = [ro (N), V (N)] -> dy/dt, N = 8,000,000.

Decomposition across 8 NeuronCores (data-parallel, no collectives):
  - Each core owns a contiguous chunk of S_OWN = 2^20 grid points of both
    ro and V (total 8*2^20 >= N; tail is zero-padded).
  - Per-core inputs carry a 2-left/1-right element halo so the 4-point TVD
    stencil is uniform everywhere; global edge cells (4 elements) and the
    firing-rate feedback (a single scalar = sum(ro*H), which only affects
    output element 0) are patched on the host from per-core partial sums.
  - Layout on core: chunk viewed as [128 partitions x LW=8192] row-major
    (partition p = contiguous segment), so the stencil is a free-axis
    shift. Tiles of width W columns, each loaded with a 3-column halo.

Math notes (exact rewrites of the reference):
  - limiter(a,b) = min(0.5|a+b|, 2min(|a|,|b|))  (the reference's masked
    sequence reduces to this because its two index sets are disjoint).
  - The quartic exp argument is factored into two quadratics so the ACT
    engine's Square(scale*x+bias) evaluates most of it.
  - exp(-T^2)/(1.00000001+erf(T)) = exp(-(T^2 + ln(1.00000001+erf(T)))).
"""
import math

import numpy as np

# ---------------- problem constants ----------------
N = 8_000_000
GL = 0.1
EL = -5.0
Cm = 0.3
IEXT = 0.4
DTS = 0.5
DT = 0.1
SQ2 = math.sqrt(2.0)
SQ2PI = 0.7978845608028654
SIGMA = 0.3 / GL * math.sqrt(0.5 * GL / Cm)
COEF = 0.5 * (1.0 - DT / DTS)            # 0.4
K = 1.0 / (SIGMA * SQ2)                  # T = K * delta_V  (= 1/sqrt(3))
CC = SQ2 * K * SQ2PI                     # g = relu(CC * dVdt)
A_CONST = -GL / Cm

# quartic p(T) = C4*T^4 + ... + C0 factored: C4*(T^2+al*T+be)(T^2+ga*T+de)
C0, C1, C2, C3, C4 = 0.0061, -1.12, -0.257, -0.072, -0.0117


def _quartic_factors():
    r = np.roots([C4, C3, C2, C1, C0])
    used = [False] * 4
    quads = []
    for i in range(4):
        if used[i]:
            continue
        ri = r[i]
        if abs(ri.imag) > 1e-12:
            for j in range(i + 1, 4):
                if not used[j] and abs(r[j] - np.conj(ri)) < 1e-8:
                    used[i] = used[j] = True
                    quads.append((-(2 * ri.real), (ri * np.conj(ri)).real))
                    break
        else:
            for j in range(i + 1, 4):
                if not used[j] and abs(r[j].imag) < 1e-12:
                    used[i] = used[j] = True
                    quads.append((-(ri + r[j]).real, (ri * r[j]).real))
                    break
    (al, be), (ga, de) = quads
    return al, be, ga, de


_AL, _BE, _GA, _DE = _quartic_factors()
AL2 = _AL / 2.0
GA2 = _GA / 2.0
E1 = _BE - _AL * _AL / 4.0
E2 = _DE - _GA * _GA / 4.0

NSCAL = 6
NCORES = 8
LW = 8192                 # row length per partition
S_OWN = 128 * LW          # 2^20 owned elements per core
TOT = NCORES * S_OWN
W = 1024                  # tile width (columns)


# ---------------- Bass program ----------------
def build_program(lw=LW, w=W):
    import concourse.bacc as bacc
    import concourse.mybir as mybir
    import concourse.tile as tile
    from concourse.tile import add_dep_helper

    AF = mybir.ActivationFunctionType
    OP = mybir.AluOpType
    F32 = mybir.dt.float32
    nt = lw // w
    assert lw % w == 0
    wa = min(lw, 2048)                     # phase-A (erf/ln) tile width
    nta = lw // wa

    nc = bacc.Bacc("TRN2", target_bir_lowering=False, debug=False)
    zin = nc.dram_tensor("zin", [2, 128, lw + 3], F32, kind="ExternalInput")
    scal = nc.dram_tensor("scal", [128, NSCAL], F32, kind="ExternalInput")
    dout = nc.dram_tensor("dout", [2, 128, lw], F32, kind="ExternalOutput")
    accout = nc.dram_tensor("accout", [128, 1], F32, kind="ExternalOutput")
    zin_ap, scal_ap = zin.ap(), scal.ap()
    zin_r = zin_ap.rearrange("q p c -> p q c")
    dout_r = dout.ap().rearrange("q p c -> p q c")
    accout_ap = accout.ap()

    with tile.TileContext(nc) as tc:
        with tc.tile_pool(name="io", bufs=2) as pio, \
             tc.tile_pool(name="tmp1", bufs=1) as p1, \
             tc.tile_pool(name="tmp2", bufs=2) as p2, \
             tc.tile_pool(name="persist", bufs=1) as pp:
            scal_sb = pp.tile([128, NSCAL], F32)
            nc.sync.dma_start(out=scal_sb[:, :], in_=scal_ap)
            negb_ap = scal_sb[:, 0:1]
            invtau_ap = scal_sb[:, 1:2]
            al2_ap = scal_sb[:, 2:3]
            ga2_ap = scal_sb[:, 3:4]
            one_ap = scal_sb[:, 4:5]
            e2_ap = scal_sb[:, 5:6]
            acc = pp.tile([128, nt], F32)
            erf_full = pp.tile([128, lw], F32)

            # preload first stencil tiles so DVE starts before phase A DMA
            z2_pre = {}
            for t in range(min(2, nt)):
                c0 = t * w
                z2 = pio.tile([128, 2, w + 3], F32, name=f"z2pre{t}")
                nc.sync.dma_start(out=z2[:, :, :], in_=zin_r[:, :, c0:c0 + w + 3])
                z2_pre[t] = z2

            # ---- phase A: all Erf ops (single act-table set) ----
            erf_insts = []
            for t in range(nta):
                a0 = t * wa
                Vt = pio.tile([128, wa], F32)
                nc.sync.dma_start(out=Vt[:, :], in_=zin_ap[1, :, a0 + 2:a0 + 2 + wa])
                bi = nc.scalar.activation(erf_full[:, a0:a0 + wa], Vt[:, :],
                                          AF.Erf, scale=-K)
                erf_insts.append(bi.ins)

            # ---- phase B1: all Ln ops, in place over erf_full ----
            ln_insts = []
            for t in range(nta):
                a0 = t * wa
                bi = nc.scalar.activation(erf_full[:, a0:a0 + wa],
                                          erf_full[:, a0:a0 + wa],
                                          AF.Ln, bias=one_ap)
                add_dep_helper(bi.ins, erf_insts[-1], sync=False,
                               reason="act-table phase order: ln after erf")
                ln_insts.append(bi.ins)

            # ---- phase B2: everything else (exp set only) ----
            for t in range(nt):
                c0 = t * w
                if t in z2_pre:
                    z2 = z2_pre[t]
                else:
                    z2 = pio.tile([128, 2, w + 3], F32, name="z2pre0")
                    nc.sync.dma_start(out=z2[:, :, :],
                                      in_=zin_r[:, :, c0:c0 + w + 3])
                Vo = z2[:, 1, 2:w + 2]
                ro_o = z2[:, 0, 2:w + 2]

                # sd[:,0]=src, sd[:,1]=-dVdt
                sd = p2.tile([128, 2, w], F32)
                nc.scalar.activation(sd[:, 1, :], Vo, AF.Identity,
                                     bias=negb_ap, scale=-A_CONST)
                T2 = p2.tile([128, w], F32)
                nc.scalar.activation(T2[:, :], Vo, AF.Square, scale=-K)
                Q1 = p2.tile([128, w], F32)
                nc.scalar.activation(Q1[:, :], Vo, AF.Square,
                                     bias=al2_ap, scale=-K)
                U2 = p2.tile([128, w], F32)
                nc.scalar.activation(U2[:, :], Vo, AF.Square,
                                     bias=ga2_ap, scale=-K)
                nc.scalar.activation(U2[:, :], U2[:, :], AF.Identity,
                                     bias=e2_ap)
                PT = Q1
                nc.vector.scalar_tensor_tensor(PT[:, :], Q1[:, :], E1, U2[:, :],
                                               OP.add, OP.mult)
                Aex = p2.tile([128, w], F32)
                bi = nc.scalar.activation(Aex[:, :], PT[:, :], AF.Exp, scale=C4)
                add_dep_helper(bi.ins, ln_insts[-1], sync=False,
                               reason="act-table phase order: exp after ln")
                r1 = T2
                nc.vector.tensor_add(r1[:, :], T2[:, :], erf_full[:, c0:c0 + w])
                Fden = p2.tile([128, w], F32)
                bi = nc.scalar.activation(Fden[:, :], r1[:, :], AF.Exp, scale=-1.0)
                add_dep_helper(bi.ins, ln_insts[-1], sync=False,
                               reason="act-table phase order: exp after ln")
                g = p2.tile([128, w], F32)
                nc.scalar.activation(g[:, :], sd[:, 1, :], AF.Relu, scale=-CC)
                m1 = g
                nc.vector.tensor_mul(m1[:, :], g[:, :], Fden[:, :])
                Hv = Aex
                nc.vector.scalar_tensor_tensor(Hv[:, :], Aex[:, :], invtau_ap,
                                               m1[:, :], OP.mult, OP.add)
                nc.vector.scalar_tensor_tensor(sd[:, 0, :], ro_o, 1.0, Hv[:, :],
                                               OP.mult, OP.mult,
                                               accum_out=acc[:, t:t + 1])

                # ---- stacked TVD stencil (ro and V together) ----
                d = p1.tile([128, 2, w + 2], F32)
                nc.vector.tensor_sub(d[:, :, :], z2[:, :, 1:w + 3],
                                     z2[:, :, 0:w + 2])
                s2 = p1.tile([128, 2, w + 1], F32)
                nc.vector.tensor_sub(s2[:, :, :], z2[:, :, 2:w + 3],
                                     z2[:, :, 0:w + 1])
                x1 = s2
                nc.scalar.activation(x1[:, :, :], s2[:, :, :], AF.Abs,
                                     scale=COEF / DTS * 0.5)
                A2 = p1.tile([128, 2, w + 2], F32)
                nc.scalar.activation(A2[:, :, :], d[:, :, :], AF.Abs,
                                     scale=COEF / DTS * 2.0)
                mA = p1.tile([128, 2, w + 1], F32)
                nc.vector.tensor_tensor(mA[:, :, :], A2[:, :, 1:w + 2],
                                        A2[:, :, 0:w + 1], OP.min)
                wi = x1
                nc.vector.tensor_tensor(wi[:, :, :], x1[:, :, :], mA[:, :, :],
                                        OP.min)
                rp = A2[:, :, 0:w]
                nc.vector.tensor_sub(rp[:, :, :], wi[:, :, 1:w + 1],
                                     wi[:, :, 0:w])
                s1 = p1.tile([128, 2, w], F32)
                nc.vector.scalar_tensor_tensor(s1[:, :, :], d[:, :, 1:w + 1],
                                               -1.0 / DTS, rp[:, :, :],
                                               OP.mult, OP.subtract)
                f = s1
                nc.vector.tensor_sub(f[:, :, :], s1[:, :, :], sd[:, :, :])
                nc.sync.dma_start(out=dout_r[:, :, c0:c0 + w], in_=f[:, :, :])

            accsum = pp.tile([128, 1], F32)
            nc.vector.tensor_reduce(accsum[:, :], acc[:, :],
                                    axis=mybir.AxisListType.X, op=OP.add)
            nc.sync.dma_start(out=accout_ap, in_=accsum[:, :])
    nc.compile()
    return nc


_NC_CACHE = {}


def _get_program(lw=LW, w=W):
    key = (lw, w)
    if key not in _NC_CACHE:
        _NC_CACHE[key] = build_program(lw, w)
    return _NC_CACHE[key]


def run_cores(ro_pad, v_pad, b_val, invtau_val, lw=LW, w=W, ncores=NCORES,
              trace=False):
    """ro_pad/v_pad: f32 arrays of length ncores*128*lw + 3 (2 left halo,
    owned, 1 right halo). Returns (out [2, ncores*128*lw], firing_partials
    [ncores,128], results_obj)."""
    from concourse.bass_utils import run_bass_kernel_spmd

    s_own = 128 * lw
    nc = _get_program(lw, w)
    scal = np.empty((128, NSCAL), np.float32)
    scal[:, 0] = -b_val
    scal[:, 1] = invtau_val
    scal[:, 2] = AL2
    scal[:, 3] = GA2
    scal[:, 4] = 1.00000001
    scal[:, 5] = E2

    in_maps = []
    for c in range(ncores):
        base = c * s_own
        zin = np.empty((2, 128, lw + 3), np.float32)
        for q, arr in ((0, ro_pad), (1, v_pad)):
            view = np.lib.stride_tricks.as_strided(
                arr[base:], shape=(128, lw + 3),
                strides=(lw * arr.itemsize, arr.itemsize))
            zin[q] = view
        in_maps.append({"zin": zin, "scal": scal})

    res = run_bass_kernel_spmd(nc, in_maps, list(range(ncores)), trace=trace)
    outs = np.empty((2, ncores * s_own), np.float32)
    partials = np.empty((ncores, 128), np.float32)
    for c in range(ncores):
        m = res.results[c]
        outs[0, c * s_own:(c + 1) * s_own] = m["dout"][0].reshape(-1)
        outs[1, c * s_own:(c + 1) * s_own] = m["dout"][1].reshape(-1)
        partials[c] = m["accout"].reshape(-1)
    return outs, partials, res


def _erf(x):
    return math.erf(x)


def _H_scalar(V, dVdt, invtau):
    f32 = np.float32
    V = f32(V)
    dVdt = f32(dVdt)
    delta_V = max(f32(-V), f32(-1.0))
    T = f32(delta_V * f32(K))
    T2 = f32(T * T)
    p = f32(C0) + f32(C1) * T + f32(C2) * T2 + f32(C3) * T2 * T \
        + f32(C4) * T2 * T2
    A = np.exp(p, dtype=f32)
    den = f32(_erf(float(T)) + 1.00000001)
    F = np.exp(f32(-T2 - np.log(den, dtype=f32)), dtype=f32)
    g = max(dVdt * f32(CC), f32(0.0))
    return f32(A * f32(invtau) + g * F)


def _limiter(a, b):
    return min(0.5 * abs(a + b), 2.0 * min(abs(a), abs(b)))


def kernel(t=None, y=None, gsyn=None, Isyn=None, **_ignored):
    f32 = np.float32
    y = np.asarray(y, f32)
    ro = y[:N]
    V = y[N:]
    Isyn_s = float(np.asarray(Isyn, f32).reshape(-1)[0])
    gsum = float(np.sum(np.asarray(gsyn, f32), dtype=f32))
    tau_m = Cm / (GL + gsum)
    invtau = 1.0 / tau_m
    b_val = (GL * EL + IEXT + Isyn_s) / Cm

    # padded inputs: [2 halo][N][pad zeros][1 halo]; left halo = dup of elem 0
    ro_pad = np.zeros(2 + TOT + 1, f32)
    ro_pad[0:2] = ro[0]
    ro_pad[2:2 + N] = ro
    v_pad = np.zeros(2 + TOT + 1, f32)
    v_pad[0:2] = V[0]
    v_pad[2:2 + N] = V

    outs, partials, _ = run_cores(ro_pad, v_pad, b_val, invtau)

    firing = f32(np.sum(partials, dtype=np.float64))
    dro = outs[0][:N]
    dV = outs[1][:N]
    # host fixups (4 edge elements)
    dro[0] = -ro[0] / f32(DTS) + firing
    wi_last = _limiter(float(ro[N - 1]) - float(ro[N - 2]),
                       float(ro[N - 2]) - float(ro[N - 3]))
    dVdt_last = f32(A_CONST) * V[N - 1] + f32(b_val)
    src_last = ro[N - 1] * _H_scalar(V[N - 1], dVdt_last, invtau)
    dro[N - 1] = (ro[N - 2] + f32(COEF) * f32(wi_last)) / f32(DTS) - src_last
    dV[0] = 0.0
    dV[N - 1] = dVdt_last
    return np.concatenate([dro, dV])


# revision 13
# speedup vs baseline: 1.4493x; 1.0406x over previous
"""Trainium2 Bass kernel for nn_Network_10256381903586.

Population-density LIF network RHS:
  y = [ro (N), V (N)] -> dy/dt, N = 8,000,000.

Decomposition across 8 NeuronCores (data-parallel, no collectives):
  - Each core owns a contiguous chunk of S_OWN = 2^20 grid points of both
    ro and V (total 8*2^20 >= N; tail is zero-padded).
  - Per-core inputs carry a 2-left/1-right element halo so the 4-point TVD
    stencil is uniform everywhere; global edge cells (4 elements) and the
    firing-rate feedback (a single scalar = sum(ro*H), which only affects
    output element 0) are patched on the host from per-core partial sums.
  - Layout on core: chunk viewed as [128 partitions x LW=8192] row-major
    (partition p = contiguous segment), so the stencil is a free-axis
    shift. Tiles of width W columns, each loaded with a 3-column halo.

Math notes (exact rewrites of the reference):
  - limiter(a,b) = min(0.5|a+b|, 2min(|a|,|b|))  (the reference's masked
    sequence reduces to this because its two index sets are disjoint).
  - The quartic exp argument is factored into two quadratics so the ACT
    engine's Square(scale*x+bias) evaluates most of it.
  - exp(-T^2)/(1.00000001+erf(T)) = exp(-(T^2 + ln(1.00000001+erf(T)))).
"""
import math

import numpy as np

# ---------------- problem constants ----------------
N = 8_000_000
GL = 0.1
EL = -5.0
Cm = 0.3
IEXT = 0.4
DTS = 0.5
DT = 0.1
SQ2 = math.sqrt(2.0)
SQ2PI = 0.7978845608028654
SIGMA = 0.3 / GL * math.sqrt(0.5 * GL / Cm)
COEF = 0.5 * (1.0 - DT / DTS)            # 0.4
K = 1.0 / (SIGMA * SQ2)                  # T = K * delta_V  (= 1/sqrt(3))
CC = SQ2 * K * SQ2PI                     # g = relu(CC * dVdt)
A_CONST = -GL / Cm

# quartic p(T) = C4*T^4 + ... + C0 factored: C4*(T^2+al*T+be)(T^2+ga*T+de)
C0, C1, C2, C3, C4 = 0.0061, -1.12, -0.257, -0.072, -0.0117


def _quartic_factors():
    r = np.roots([C4, C3, C2, C1, C0])
    used = [False] * 4
    quads = []
    for i in range(4):
        if used[i]:
            continue
        ri = r[i]
        if abs(ri.imag) > 1e-12:
            for j in range(i + 1, 4):
                if not used[j] and abs(r[j] - np.conj(ri)) < 1e-8:
                    used[i] = used[j] = True
                    quads.append((-(2 * ri.real), (ri * np.conj(ri)).real))
                    break
        else:
            for j in range(i + 1, 4):
                if not used[j] and abs(r[j].imag) < 1e-12:
                    used[i] = used[j] = True
                    quads.append((-(ri + r[j]).real, (ri * r[j]).real))
                    break
    (al, be), (ga, de) = quads
    return al, be, ga, de


_AL, _BE, _GA, _DE = _quartic_factors()
AL2 = _AL / 2.0
GA2 = _GA / 2.0
E1 = _BE - _AL * _AL / 4.0
E2 = _DE - _GA * _GA / 4.0

NSCAL = 6
NCORES = 8
LW = 8192                 # row length per partition
S_OWN = 128 * LW          # 2^20 owned elements per core
TOT = NCORES * S_OWN
W = 1024                  # tile width (columns)


# ---------------- Bass program ----------------
def build_program(lw=LW, w=W):
    import concourse.bacc as bacc
    import concourse.mybir as mybir
    import concourse.tile as tile
    from concourse.tile import add_dep_helper

    AF = mybir.ActivationFunctionType
    OP = mybir.AluOpType
    F32 = mybir.dt.float32
    nt = lw // w
    assert lw % w == 0
    wa = min(lw, 2048)                     # phase-A (erf/ln) tile width
    nta = lw // wa

    nc = bacc.Bacc("TRN2", target_bir_lowering=False, debug=False)
    zin = nc.dram_tensor("zin", [2, 128, lw + 3], F32, kind="ExternalInput")
    scal = nc.dram_tensor("scal", [128, NSCAL], F32, kind="ExternalInput")
    dout = nc.dram_tensor("dout", [2, 128, lw], F32, kind="ExternalOutput")
    accout = nc.dram_tensor("accout", [128, 1], F32, kind="ExternalOutput")
    zin_ap, scal_ap = zin.ap(), scal.ap()
    zin_r = zin_ap.rearrange("q p c -> p q c")
    dout_r = dout.ap().rearrange("q p c -> p q c")
    accout_ap = accout.ap()

    with tile.TileContext(nc) as tc:
        with tc.tile_pool(name="io", bufs=2) as pio, \
             tc.tile_pool(name="tmp1", bufs=1) as p1, \
             tc.tile_pool(name="tmp2", bufs=2) as p2, \
             tc.tile_pool(name="persist", bufs=1) as pp:
            scal_sb = pp.tile([128, NSCAL], F32)
            nc.sync.dma_start(out=scal_sb[:, :], in_=scal_ap)
            negb_ap = scal_sb[:, 0:1]
            invtau_ap = scal_sb[:, 1:2]
            al2_ap = scal_sb[:, 2:3]
            ga2_ap = scal_sb[:, 3:4]
            one_ap = scal_sb[:, 4:5]
            e2_ap = scal_sb[:, 5:6]
            acc = pp.tile([128, nt], F32)
            erf_full = pp.tile([128, lw], F32)

            # preload first stencil tiles so DVE starts before phase A DMA
            z2_pre = {}
            for t in range(min(2, nt)):
                c0 = t * w
                z2 = pio.tile([128, 2, w + 3], F32, name=f"z2pre{t}")
                nc.sync.dma_start(out=z2[:, :, :], in_=zin_r[:, :, c0:c0 + w + 3])
                z2_pre[t] = z2

            # ---- phase A: all Erf ops (single act-table set) ----
            erf_insts = []
            for t in range(nta):
                a0 = t * wa
                Vt = pio.tile([128, wa], F32)
                nc.sync.dma_start(out=Vt[:, :], in_=zin_ap[1, :, a0 + 2:a0 + 2 + wa])
                bi = nc.scalar.activation(erf_full[:, a0:a0 + wa], Vt[:, :],
                                          AF.Erf, scale=-K)
                erf_insts.append(bi.ins)

            # ---- phase B1: all Ln ops, in place over erf_full ----
            ln_insts = []
            for t in range(nta):
                a0 = t * wa
                bi = nc.scalar.activation(erf_full[:, a0:a0 + wa],
                                          erf_full[:, a0:a0 + wa],
                                          AF.Ln, bias=one_ap)
                add_dep_helper(bi.ins, erf_insts[-1], sync=False,
                               reason="act-table phase order: ln after erf")
                ln_insts.append(bi.ins)

            # ---- phase B2: everything else (exp set only) ----
            for t in range(nt):
                c0 = t * w
                if t in z2_pre:
                    z2 = z2_pre[t]
                else:
                    z2 = pio.tile([128, 2, w + 3], F32, name="z2pre0")
                    nc.sync.dma_start(out=z2[:, :, :],
                                      in_=zin_r[:, :, c0:c0 + w + 3])
                Vo = z2[:, 1, 2:w + 2]
                ro_o = z2[:, 0, 2:w + 2]

                # sd[:,0]=src, sd[:,1]=-dVdt
                sd = p2.tile([128, 2, w], F32)
                nc.scalar.activation(sd[:, 1, :], Vo, AF.Identity,
                                     bias=negb_ap, scale=-A_CONST)
                T2 = p2.tile([128, w], F32)
                nc.scalar.activation(T2[:, :], Vo, AF.Square, scale=-K)
                Q1 = p2.tile([128, w], F32)
                nc.scalar.activation(Q1[:, :], Vo, AF.Square,
                                     bias=al2_ap, scale=-K)
                U2 = p2.tile([128, w], F32)
                nc.scalar.activation(U2[:, :], Vo, AF.Square,
                                     bias=ga2_ap, scale=-K)
                nc.scalar.activation(U2[:, :], U2[:, :], AF.Identity,
                                     bias=e2_ap)
                PT = Q1
                nc.vector.scalar_tensor_tensor(PT[:, :], Q1[:, :], E1, U2[:, :],
                                               OP.add, OP.mult)
                Aex = p2.tile([128, w], F32)
                bi = nc.scalar.activation(Aex[:, :], PT[:, :], AF.Exp, scale=C4)
                add_dep_helper(bi.ins, ln_insts[-1], sync=False,
                               reason="act-table phase order: exp after ln")
                r1 = T2
                nc.vector.tensor_add(r1[:, :], T2[:, :], erf_full[:, c0:c0 + w])
                Fden = p2.tile([128, w], F32)
                bi = nc.scalar.activation(Fden[:, :], r1[:, :], AF.Exp, scale=-1.0)
                add_dep_helper(bi.ins, ln_insts[-1], sync=False,
                               reason="act-table phase order: exp after ln")
                g = p2.tile([128, w], F32)
                nc.scalar.activation(g[:, :], sd[:, 1, :], AF.Relu, scale=-CC)
                m1 = g
                nc.vector.tensor_mul(m1[:, :], g[:, :], Fden[:, :])
                Hv = Aex
                nc.vector.scalar_tensor_tensor(Hv[:, :], Aex[:, :], invtau_ap,
                                               m1[:, :], OP.mult, OP.add)
                nc.vector.scalar_tensor_tensor(sd[:, 0, :], ro_o, 1.0, Hv[:, :],
                                               OP.mult, OP.mult,
                                               accum_out=acc[:, t:t + 1])

                # ---- stacked TVD stencil (ro and V together) ----
                d = p2.tile([128, 2, w + 2], F32)
                nc.vector.tensor_sub(d[:, :, :], z2[:, :, 1:w + 3],
                                     z2[:, :, 0:w + 2])
                s2 = p1.tile([128, 2, w + 1], F32)
                nc.vector.tensor_sub(s2[:, :, :], z2[:, :, 2:w + 3],
                                     z2[:, :, 0:w + 1])
                x1 = s2
                nc.scalar.activation(x1[:, :, :], s2[:, :, :], AF.Abs,
                                     scale=COEF / DTS * 0.5)
                A2 = p1.tile([128, 2, w + 2], F32)
                nc.scalar.activation(A2[:, :, :], d[:, :, :], AF.Abs,
                                     scale=COEF / DTS * 2.0)
                mA = p1.tile([128, 2, w + 1], F32)
                nc.vector.tensor_tensor(mA[:, :, :], A2[:, :, 1:w + 2],
                                        A2[:, :, 0:w + 1], OP.min)
                wi = x1
                nc.vector.tensor_tensor(wi[:, :, :], x1[:, :, :], mA[:, :, :],
                                        OP.min)
                rp = A2[:, :, 0:w]
                nc.vector.tensor_sub(rp[:, :, :], wi[:, :, 1:w + 1],
                                     wi[:, :, 0:w])
                s1 = p2.tile([128, 2, w], F32)
                nc.vector.scalar_tensor_tensor(s1[:, :, :], d[:, :, 1:w + 1],
                                               -1.0 / DTS, rp[:, :, :],
                                               OP.mult, OP.subtract)
                f = s1
                nc.vector.tensor_sub(f[:, :, :], s1[:, :, :], sd[:, :, :])
                nc.sync.dma_start(out=dout_r[:, :, c0:c0 + w], in_=f[:, :, :])

            accsum = pp.tile([128, 1], F32)
            nc.vector.tensor_reduce(accsum[:, :], acc[:, :],
                                    axis=mybir.AxisListType.X, op=OP.add)
            nc.sync.dma_start(out=accout_ap, in_=accsum[:, :])
    nc.compile()
    return nc


_NC_CACHE = {}


def _get_program(lw=LW, w=W):
    key = (lw, w)
    if key not in _NC_CACHE:
        _NC_CACHE[key] = build_program(lw, w)
    return _NC_CACHE[key]


def run_cores(ro_pad, v_pad, b_val, invtau_val, lw=LW, w=W, ncores=NCORES,
              trace=False):
    """ro_pad/v_pad: f32 arrays of length ncores*128*lw + 3 (2 left halo,
    owned, 1 right halo). Returns (out [2, ncores*128*lw], firing_partials
    [ncores,128], results_obj)."""
    from concourse.bass_utils import run_bass_kernel_spmd

    s_own = 128 * lw
    nc = _get_program(lw, w)
    scal = np.empty((128, NSCAL), np.float32)
    scal[:, 0] = -b_val
    scal[:, 1] = invtau_val
    scal[:, 2] = AL2
    scal[:, 3] = GA2
    scal[:, 4] = 1.00000001
    scal[:, 5] = E2

    in_maps = []
    for c in range(ncores):
        base = c * s_own
        zin = np.empty((2, 128, lw + 3), np.float32)
        for q, arr in ((0, ro_pad), (1, v_pad)):
            view = np.lib.stride_tricks.as_strided(
                arr[base:], shape=(128, lw + 3),
                strides=(lw * arr.itemsize, arr.itemsize))
            zin[q] = view
        in_maps.append({"zin": zin, "scal": scal})

    res = run_bass_kernel_spmd(nc, in_maps, list(range(ncores)), trace=trace)
    outs = np.empty((2, ncores * s_own), np.float32)
    partials = np.empty((ncores, 128), np.float32)
    for c in range(ncores):
        m = res.results[c]
        outs[0, c * s_own:(c + 1) * s_own] = m["dout"][0].reshape(-1)
        outs[1, c * s_own:(c + 1) * s_own] = m["dout"][1].reshape(-1)
        partials[c] = m["accout"].reshape(-1)
    return outs, partials, res


def _erf(x):
    return math.erf(x)


def _H_scalar(V, dVdt, invtau):
    f32 = np.float32
    V = f32(V)
    dVdt = f32(dVdt)
    delta_V = max(f32(-V), f32(-1.0))
    T = f32(delta_V * f32(K))
    T2 = f32(T * T)
    p = f32(C0) + f32(C1) * T + f32(C2) * T2 + f32(C3) * T2 * T \
        + f32(C4) * T2 * T2
    A = np.exp(p, dtype=f32)
    den = f32(_erf(float(T)) + 1.00000001)
    F = np.exp(f32(-T2 - np.log(den, dtype=f32)), dtype=f32)
    g = max(dVdt * f32(CC), f32(0.0))
    return f32(A * f32(invtau) + g * F)


def _limiter(a, b):
    return min(0.5 * abs(a + b), 2.0 * min(abs(a), abs(b)))


def kernel(t=None, y=None, gsyn=None, Isyn=None, **_ignored):
    f32 = np.float32
    y = np.asarray(y, f32)
    ro = y[:N]
    V = y[N:]
    Isyn_s = float(np.asarray(Isyn, f32).reshape(-1)[0])
    gsum = float(np.sum(np.asarray(gsyn, f32), dtype=f32))
    tau_m = Cm / (GL + gsum)
    invtau = 1.0 / tau_m
    b_val = (GL * EL + IEXT + Isyn_s) / Cm

    # padded inputs: [2 halo][N][pad zeros][1 halo]; left halo = dup of elem 0
    ro_pad = np.zeros(2 + TOT + 1, f32)
    ro_pad[0:2] = ro[0]
    ro_pad[2:2 + N] = ro
    v_pad = np.zeros(2 + TOT + 1, f32)
    v_pad[0:2] = V[0]
    v_pad[2:2 + N] = V

    outs, partials, _ = run_cores(ro_pad, v_pad, b_val, invtau)

    firing = f32(np.sum(partials, dtype=np.float64))
    dro = outs[0][:N]
    dV = outs[1][:N]
    # host fixups (4 edge elements)
    dro[0] = -ro[0] / f32(DTS) + firing
    wi_last = _limiter(float(ro[N - 1]) - float(ro[N - 2]),
                       float(ro[N - 2]) - float(ro[N - 3]))
    dVdt_last = f32(A_CONST) * V[N - 1] + f32(b_val)
    src_last = ro[N - 1] * _H_scalar(V[N - 1], dVdt_last, invtau)
    dro[N - 1] = (ro[N - 2] + f32(COEF) * f32(wi_last)) / f32(DTS) - src_last
    dV[0] = 0.0
    dV[N - 1] = dVdt_last
    return np.concatenate([dro, dV])
